# revision 56
# baseline (speedup 1.0000x reference)
"""GPNN message-passing kernel for 8x Trainium2 NeuronCores.

Strategy:
  - Pure data parallel over batch: B=16 graphs -> 2 graphs per core.
  - Feature-major layout on chip: activations stored [feat_partitions, edges].
  - float32r matmuls (full PE rate at N>=512, ~2e-4 rel err).
  - Edge-state parity trick: E_{t+1}[w,v] = msg_t[v,w] is stored in place
    (no physical transpose); even/odd iterations flip the interpretation of
    the two N axes.  Invalid-pair entries of E are dead values (masked out of
    every output path), so they may hold garbage.
  - w-validity masking folded into hn via -1e30 additive mask before relu.
  - gate broadcast across partitions via a K=1 ones-matmul into PSUM,
    fused relu+gate-multiply via scalar_tensor_tensor.
"""

import os
import sys

for _p in ("/opt/trn_rl_repo",):
    if _p not in sys.path:
        sys.path.insert(0, _p)

import numpy as np

B, N, NF, EF, M, HID = 16, 64, 1024, 256, 256, 512
L_PROP = 3
NCORES = 8
BG = B // NCORES          # graphs per core = 2
NE = N * N                # 4096 edges per graph
ECH = 512                 # edges per chunk
NCH = NE // ECH           # 8 chunks
VCH = ECH // N            # 8 outer-axis rows per chunk
NEG = -1.0e30
A1_WINDOW = 2             # chunks of a1 emitted ahead
NBIAS = 2 + 2 + 4 + 4 + 1 + 2 + 4 + 2 + 2 + 12 + 3   # packed bias columns = 38

_CACHE = {}


def _build_program():
    import concourse.bass as bass
    import concourse.tile as tile
    from concourse import bacc, mybir

    F32R = mybir.dt.float32r
    FP32 = mybir.dt.float32
    AT = mybir.ActivationFunctionType
    OP = mybir.AluOpType
    AX = mybir.AxisListType

    nc = bacc.Bacc("TRN2", target_bir_lowering=False, debug=False,
                   num_devices=NCORES)

    def din(name, shape, dt=FP32):
        return nc.dram_tensor(name, list(shape), dt, kind="ExternalInput").ap()

    def dout(name, shape, dt=FP32):
        return nc.dram_tensor(name, list(shape), dt, kind="ExternalOutput").ap()

    # ---- inputs ----
    rsf = din("rsf", [BG, NE, EF])               # relative_spatial_feature
    cnf = din("cnf", [BG, N, NF])                # concatenated_node_features
    # weights (pre-transposed host-side to [K, M] layout), f32r
    wer_t = din("wer_t", [EF, M], F32R)
    wnr_t = din("wnr_t", [NF, M], F32R)
    w1_t = din("w1_t", [M, HID], F32R)
    w2_t = din("w2_t", [HID, HID], F32R)
    w3_t = din("w3_t", [HID, 1], F32R)
    wmn_t = din("wmn_t", [M, M], F32R)
    wme_t = din("wme_t", [M, M], F32R)
    wih_t = din("wih_t", [M, 3 * M], F32R)
    whh_t = din("whh_t", [M, 3 * M], F32R)
    cls_w1 = {k: din(f"{k}1_t", [M, HID], F32R) for k in ("lr", "cr", "mr")}
    cls_od = {"lr": 4, "cr": 6, "mr": 17}
    cls_w2 = {k: din(f"{k}2_t", [HID, cls_od[k]], F32R) for k in cls_od}
    gmat = din("gmat", [BG, N, 128], F32R)       # pair-gather matrix
    ones = din("ones", [1, 128], F32R)
    ident = din("ident", [128, 128])
    # all biases packed column-wise: [128, NBIAS]
    bias_all = din("bias_all", [128, NBIAS])
    # masks
    hnmask = din("hnmask", [BG, 128, N])         # 0 / -1e30 along w
    maskv = din("maskv", [BG, 128, N])           # 1.0 / 0.0 along v
    mask2 = din("mask2", [BG, NE])               # valid pair mask flat

    # ---- outputs ----
    out_lr = dout("out_lr", [BG, 128, 4])
    out_cr = dout("out_cr", [BG, 128, 6])
    out_mr = dout("out_mr", [BG, 128, 17])
    out_pa = dout("out_pa", [BG, NE])

    KM, KH, KNF = M // 128, HID // 128, NF // 128  # 2, 4, 8

    with tile.TileContext(nc, trace_sim=bool(os.environ.get("KTRACE"))) as tc:
        from contextlib import ExitStack
        ctx = ExitStack()
        with ctx:
            wp = ctx.enter_context(tc.tile_pool(name="wp", bufs=1))
            sb = ctx.enter_context(tc.tile_pool(name="sb", bufs=1))
            ps_mm = ctx.enter_context(tc.tile_pool(name="ps_mm", bufs=4, space="PSUM"))
            ps_pd = ctx.enter_context(tc.tile_pool(name="ps_pd", bufs=1, space="PSUM"))
            ps_sm = ctx.enter_context(tc.tile_pool(name="ps_sm", bufs=2, space="PSUM"))

            _wq = [nc.sync, nc.scalar]
            _wqi = [0]

            def wtile(src, k, width, dt=F32R, pfx="w"):
                t = wp.tile([128, width], dt, name=f"{pfx}_{src.name}_{k}")
                _wq[_wqi[0] % 2].dma_start(t[:], src[k * 128:(k + 1) * 128, :])
                _wqi[0] += 1
                return t

            bias_sb = wp.tile([128, NBIAS], FP32, name="bias_sb")
            nc.sync.dma_start(bias_sb[:], bias_all)
            nf_pre = {}
            nf_pre[0] = sb.tile([N, NF], FP32, tag="nf_nm", bufs=1, name="nfnm_pre0")
            nc.sync.dma_start(nf_pre[0][:], cnf[0])
            _bcol = iter(range(NBIAS))
            def bslice(rows=128):
                i = next(_bcol)
                return bias_sb[0:rows, i:i + 1]
            bedge_sb = [bslice() for k in range(KM)]
            bnode_sb = [bslice() for k in range(KM)]
            b1_sb = [bslice() for k in range(KH)]
            b2_sb = [bslice() for k in range(KH)]
            b3_sb = bslice(1)
            bm_sb = [bslice() for k in range(KM)]
            brz_sb = [bslice() for k in range(2 * M // 128)]
            bihn_sb = [bslice() for k in range(KM)]
            bhhn_sb = [bslice() for k in range(KM)]
            cb1_sb = {k: [bslice() for j in range(KH)] for k in cls_od}
            cb2_sb = {k: bslice(cls_od[k]) for k in cls_od}
            # ---- load weights (ordered by first use, spread over 2 rings) ----
            id_sb = wp.tile([128, 128], FP32, name="id_sb")
            nc.scalar.dma_start(id_sb[:], ident)
            wer_sb = [wtile(wer_t, k, M) for k in range(KM)]
            wme_sb = [wtile(wme_t, k, M) for k in range(KM)]
            wnr_sb = [wtile(wnr_t, k, M) for k in range(KNF)]
            w1_sb = [wtile(w1_t, k, HID) for k in range(KM)]
            w2_sb = [wtile(w2_t, k, HID) for k in range(KH)]
            w3_sb = [wtile(w3_t, k, 1) for k in range(KH)]
            wmn_sb = [wtile(wmn_t, k, M) for k in range(KM)]
            ones_sb = wp.tile([1, 128], F32R, name="ones_sb")
            nc.scalar.dma_start(ones_sb[:], ones)



            MMOP = dict(start=True, stop=True)
            outmap = {"lr": out_lr, "cr": out_cr, "mr": out_mr}
            H_final = [None] * BG
            ci_sb = [sb.tile([128, BG * 128], F32R, name=f"ci_{k}") for k in range(KM)]

            def make_graph_state(g):
                """Emit phase 0 (node features, H0, masks) and return per-graph
                state with phase-1 emitters."""
                st = {}
                if g in nf_pre:
                    nf_nm = nf_pre[g]
                else:
                    nf_nm = sb.tile([N, NF], FP32, tag="nf_nm", bufs=1, name=f"nfnm{g}")
                    nc.sync.dma_start(nf_nm[:], cnf[g])
                nf_fm = []
                for k8 in range(KNF):
                    pt = ps_mm.tile([128, N], FP32, tag="mm", name=f"nft{g}_{k8}")
                    nc.tensor.transpose(pt[:], nf_nm[:, k8 * 128:(k8 + 1) * 128],
                                        id_sb[0:N, 0:N])
                    t_ = sb.tile([128, N], F32R, tag="nf_fm", bufs=8,
                                 name=f"nffm{g}_{k8}")
                    nc.vector.tensor_copy(t_[:], pt[:])
                    nf_fm.append(t_)
                H_cur = []
                for j in range(KM):
                    ph = ps_mm.tile([128, N], FP32, tag="mm", name=f"h0p{g}_{j}")
                    for k in range(KNF):
                        nc.tensor.matmul(ph[:], wnr_sb[k][:, j * 128:(j + 1) * 128],
                                         nf_fm[k][:], start=(k == 0), stop=(k == KNF - 1))
                    ht = sb.tile([128, N], F32R, tag=f"H{g}_{j}", bufs=2,
                                 name=f"H0_{g}_{j}")
                    nc.scalar.activation(ht[:], ph[:], AT.Identity, bias=bnode_sb[j][:])
                    H_cur.append(ht)
                st["H"] = H_cur
                hnm_sb = sb.tile([128, N], FP32, tag="hnm", bufs=2, name=f"hnm{g}")
                nc.sync.dma_start(hnm_sb[:], hnmask[g])
                mv_sb = sb.tile([128, N], FP32, tag="mv", bufs=2, name=f"mv{g}")
                nc.sync.dma_start(mv_sb[:], maskv[g])
                st["hnm"], st["mv"] = hnm_sb, mv_sb

                E_t = [[None] * NCH for _ in range(KM)]
                en_big = [sb.tile([128, NE], FP32, tag=f"en{j}", name=f"en{g}_{j}")
                          for j in range(KM)]
                st["E"], st["en"] = E_t, en_big
                a1_pre = [None] * NCH
                st["a1_pre"] = a1_pre

                def emit_a1g(c, sfx):
                    tiles = []
                    for j in range(KH):
                        pa = ps_mm.tile([128, ECH], FP32, tag="mm",
                                        name=f"a1p{g}_{sfx}_{c}_{j}")
                        for k in range(KM):
                            nc.tensor.matmul(pa[:], w1_sb[k][:, j * 128:(j + 1) * 128],
                                             E_t[k][c][:], start=(k == 0),
                                             stop=(k == KM - 1))
                        at = sb.tile([128, ECH], F32R, tag="a1", bufs=4 * A1_WINDOW + 2,
                                     name=f"a1{g}_{sfx}_{c}_{j}")
                        nc.scalar.activation(at[:], pa[:], AT.Relu, bias=b1_sb[j][:])
                        tiles.append(at)
                    return tiles
                st["emit_a1g"] = emit_a1g

                def ph1a(c, interleaved=False):
                    tp_pool, tp_tag = (ps_sm, "sm") if interleaved else (ps_mm, "mm")
                    rf_fm = [sb.tile([128, ECH], F32R, tag=f"rf{j}", bufs=2,
                                     name=f"rffm{g}_{c}_{j}") for j in range(KM)]
                    rnm = sb.tile([128, 4 * EF], FP32, tag="rnm", bufs=2,
                                  name=f"rnm{g}_{c}")
                    nc.sync.dma_start(
                        rnm[:].rearrange("p (et f) -> p et f", et=4),
                        rsf[g, c * ECH:(c + 1) * ECH, :].rearrange(
                            "(et p) f -> p et f", et=4))
                    for et in range(ECH // 128):
                        for j in range(KM):
                            pt = tp_pool.tile([128, 128], FP32, tag=tp_tag,
                                              name=f"rt{g}_{c}_{et}_{j}")
                            o0 = et * EF + j * 128
                            nc.tensor.transpose(pt[:], rnm[:, o0:o0 + 128],
                                                id_sb[:, :])
                            if (et + j) % 2 == 0:
                                nc.vector.tensor_copy(
                                    rf_fm[j][:, et * 128:(et + 1) * 128], pt[:])
                            else:
                                nc.scalar.copy(
                                    rf_fm[j][:, et * 128:(et + 1) * 128], pt[:])
                    for j in range(KM):
                        pe0 = ps_mm.tile([128, ECH], FP32, tag="mm",
                                         name=f"e0p{g}_{c}_{j}")
                        for h in range(2):
                            hs = slice(h * 256, (h + 1) * 256)
                            for k in range(KM):
                                nc.tensor.matmul(pe0[:, hs],
                                                 wer_sb[k][:, j * 128:(j + 1) * 128],
                                                 rf_fm[k][:, hs],
                                                 start=(k == 0), stop=(k == KM - 1))
                        et_ = sb.tile([128, ECH], F32R, tag=f"E{j}_{c}",
                                      name=f"E{g}_{j}_{c}")
                        nc.scalar.activation(et_[:], pe0[:], AT.Identity,
                                             bias=bedge_sb[j][:])
                        E_t[j][c] = et_
                    if c < A1_WINDOW:
                        a1_pre[c] = emit_a1g(c, "p")
                st["ph1a"] = ph1a

                def ph1b(c):
                    for j in range(KM):
                        pen = ps_mm.tile([128, ECH], FP32, tag="mm",
                                         name=f"enp{g}_{c}_{j}")
                        for k in range(KM):
                            nc.tensor.matmul(pen[:], wme_sb[k][:, j * 128:(j + 1) * 128],
                                             E_t[k][c][:], start=(k == 0),
                                             stop=(k == KM - 1))
                        nc.vector.tensor_copy(en_big[j][:, c * ECH:(c + 1) * ECH],
                                              pen[:])
                st["ph1b"] = ph1b
                return st

            states = [None] * BG
            states[0] = make_graph_state(0)
            for c in range(NCH):
                states[0]["ph1a"](c)
                states[0]["ph1b"](c)

            # late-needed weights: load during phase 1 compute
            wih_sb = [wtile(wih_t, k, 3 * M) for k in range(KM)]
            whh_sb = [wtile(whh_t, k, 3 * M) for k in range(KM)]
            c1_sb = {k: [wtile(cls_w1[k], j, HID) for j in range(KM)] for k in cls_od}
            c2_sb = {k: [wtile(cls_w2[k], j, cls_od[k]) for j in range(KH)]
                     for k in cls_od}

            for g in range(BG):
                st = states[g]
                E_t, en_big = st["E"], st["en"]
                H_cur, hnm_sb, mv_sb = st["H"], st["hnm"], st["mv"]
                a1_pre, emit_a1g = st["a1_pre"], st["emit_a1g"]
                last = (g == BG - 1)

                for t in range(L_PROP):
                    even = (t % 2 == 0)
                    final_t = (t == L_PROP - 1)
                    hn_m = []
                    for j in range(KM):
                        phn = ps_sm.tile([128, N], FP32, tag="sm", name=f"hnp{g}_{t}_{j}")
                        for k in range(KM):
                            nc.tensor.matmul(phn[:], wmn_sb[k][:, j * 128:(j + 1) * 128],
                                             H_cur[k][:], start=(k == 0), stop=(k == KM - 1))
                        hnt = sb.tile([128, N], FP32, tag="hn", bufs=4,
                                      name=f"hn{g}_{t}_{j}")
                        nc.scalar.activation(hnt[:], phn[:], AT.Identity, bias=bm_sb[j][:])
                        hm = sb.tile([128, N], FP32, tag="hnm2", bufs=4,
                                     name=f"hnm{g}_{t}_{j}")
                        nc.vector.tensor_tensor(hm[:], hnt[:], hnm_sb[:], op=OP.add)
                        hn_m.append(hm)

                    a1_t = [None] * NCH
                    m_fin = [sb.tile([128, N], F32R, tag=f"ms{j}", bufs=2,
                                     name=f"msum{g}_{t}_{j}") for j in range(KM)]
                    mps = None
                    if not even:
                        mps = [sb.tile([128, NCH * N], FP32, tag=f"mps{j}", bufs=1,
                                       name=f"mps{g}_{t}_{j}") for j in range(KM)]
                    if t == 0:
                        for c0 in range(A1_WINDOW):
                            a1_t[c0] = a1_pre[c0]
                    else:
                        for c0 in range(A1_WINDOW):
                            a1_t[c0] = prefetched_a1[c0]

                    prefetched_next = [None] * NCH
                    for c in range(NCH):
                        if c + A1_WINDOW < NCH:
                            a1_t[c + A1_WINDOW] = emit_a1g(c + A1_WINDOW, f"t{t}")
                        # a2
                        a2_t = []
                        for j in range(KH):
                            pa = ps_mm.tile([128, ECH], FP32, tag="mm",
                                            name=f"a2p{g}_{t}_{c}_{j}")
                            for k in range(KH):
                                nc.tensor.matmul(pa[:], w2_sb[k][:, j * 128:(j + 1) * 128],
                                                 a1_t[c][k][:], start=(k == 0), stop=(k == KH - 1))
                            at = sb.tile([128, ECH], F32R, tag="a2", bufs=4,
                                         name=f"a2{g}_{t}_{c}_{j}")
                            if j < 2:
                                nc.scalar.activation(at[:], pa[:], AT.Relu, bias=b2_sb[j][:])
                            else:
                                nc.vector.tensor_scalar(at[:], pa[:], b2_sb[j][:], 0.0,
                                                        op0=OP.add, op1=OP.max)
                            a2_t.append(at)
                        a1_t[c] = None
                        # padj
                        ppd = ps_pd.tile([1, ECH], FP32, tag="pd", name=f"pdp{g}_{t}_{c}")
                        for k in range(KH):
                            nc.tensor.matmul(ppd[:], w3_sb[k][:], a2_t[k][:],
                                             start=(k == 0), stop=(k == KH - 1))
                        gate = sb.tile([1, ECH], F32R, tag="gate", bufs=2,
                                       name=f"gate{g}_{t}_{c}")
                        nc.scalar.activation(gate[:], ppd[:], AT.Sigmoid, bias=b3_sb[:])
                        if final_t:
                            m2t = sb.tile([1, ECH], FP32, tag="m2", bufs=1,
                                          name=f"m2{g}_{c}")
                            nc.sync.dma_start(
                                m2t[:], mask2[g, c * ECH:(c + 1) * ECH].unsqueeze(0))
                            pam = sb.tile([1, ECH], FP32, tag="pam", bufs=1,
                                          name=f"pam{g}_{c}")
                            nc.vector.scalar_tensor_tensor(
                                pam[:], ppd[:], b3_sb[:], m2t[:],
                                op0=OP.add, op1=OP.mult)
                            nc.sync.dma_start(
                                out_pa[g, c * ECH:(c + 1) * ECH].unsqueeze(0), pam[:])
                        pgb = ps_sm.tile([128, ECH], FP32, tag="gb", bufs=1,
                                         name=f"gb{g}_{t}_{c}")
                        nc.tensor.matmul(pgb[:], ones_sb[:], gate[:], **MMOP)
                        for j in range(KM):
                            tmp = sb.tile([128, ECH], FP32, tag="tmp", bufs=2,
                                          name=f"tmp{g}_{t}_{c}_{j}")
                            tmp3 = tmp[:].rearrange("p (a b) -> p a b", a=VCH)
                            if even:
                                en_v = en_big[j][:, c * ECH:(c + 1) * ECH].rearrange(
                                    "p (a b) -> p a b", a=VCH)
                                hn_v = hn_m[j][:].unsqueeze(1).broadcast_to((128, VCH, N))
                            else:
                                en_v = en_big[j][:].rearrange(
                                    "p (v w) -> p v w", v=N).transpose([0, 2, 1])[
                                    :, c * VCH:(c + 1) * VCH, :]
                                hn_v = hn_m[j][:, c * VCH:(c + 1) * VCH].unsqueeze(
                                    2).broadcast_to((128, VCH, N))
                            nc.vector.tensor_tensor(tmp3, en_v, hn_v, op=OP.add)
                            mdst = tmp if final_t else E_t[j][c]
                            nc.vector.scalar_tensor_tensor(
                                mdst[:], tmp[:], 0.0, pgb[:],
                                op0=OP.max, op1=OP.mult)
                            with nc.allow_low_precision(reason="f32r msum"):
                                if even:
                                    nc.vector.tensor_reduce(
                                        m_fin[j][:, c * VCH:(c + 1) * VCH],
                                        mdst[:].rearrange("p (a b) -> p a b", a=VCH),
                                        axis=AX.X, op=OP.add)
                                else:
                                    nc.vector.tensor_reduce(
                                        mps[j][:, c * N:(c + 1) * N],
                                        mdst[:].rearrange("p (a b) -> p b a", a=VCH),
                                        axis=AX.X, op=OP.add)

                    if t + 1 < L_PROP:
                        for c0 in range(A1_WINDOW):
                            prefetched_next[c0] = emit_a1g(c0, f"t{t + 1}n")
                        prefetched_a1 = prefetched_next
                    if final_t and not last:
                        states[g + 1] = make_graph_state(g + 1)
                        for c_ in range(4):
                            states[g + 1]["ph1a"](c_)
                    if not even:
                        with nc.allow_low_precision(reason="f32r msum"):
                            for j in range(KM):
                                nc.vector.tensor_reduce(
                                    m_fin[j][:],
                                    mps[j][:].rearrange("p (c b) -> p b c", c=NCH),
                                    axis=AX.X, op=OP.add)
                    # ---- GRU ----
                    rz = []
                    for j in range(4):
                        prz = ps_sm.tile([128, N], FP32, tag="sm", name=f"rz{g}_{t}_{j}")
                        for k in range(KM):
                            nc.tensor.matmul(prz[:], wih_sb[k][:, j * 128:(j + 1) * 128],
                                             m_fin[k][:], start=(k == 0), stop=False)
                        for k in range(KM):
                            nc.tensor.matmul(prz[:], whh_sb[k][:, j * 128:(j + 1) * 128],
                                             H_cur[k][:], start=False, stop=(k == KM - 1))
                        rzt = sb.tile([128, N], FP32, tag="rz", bufs=6, name=f"rzt{g}_{t}_{j}")
                        nc.scalar.activation(rzt[:], prz[:], AT.Sigmoid, bias=brz_sb[j][:])
                        rz.append(rzt)
                    H_new = []
                    for j in range(KM):
                        jj = 4 + j
                        pgi = ps_sm.tile([128, N], FP32, tag="sm", name=f"gin{g}_{t}_{j}")
                        for k in range(KM):
                            nc.tensor.matmul(pgi[:], wih_sb[k][:, jj * 128:(jj + 1) * 128],
                                             m_fin[k][:], start=(k == 0), stop=(k == KM - 1))
                        pgh = ps_sm.tile([128, N], FP32, tag="sm", name=f"ghn{g}_{t}_{j}")
                        for k in range(KM):
                            nc.tensor.matmul(pgh[:], whh_sb[k][:, jj * 128:(jj + 1) * 128],
                                             H_cur[k][:], start=(k == 0), stop=(k == KM - 1))
                        s1 = sb.tile([128, N], FP32, tag="s1", bufs=2, name=f"s1{g}_{t}_{j}")
                        nc.scalar.activation(s1[:], pgh[:], AT.Identity, bias=bhhn_sb[j][:])
                        s2 = sb.tile([128, N], FP32, tag="s2", bufs=2, name=f"s2{g}_{t}_{j}")
                        nc.vector.tensor_tensor(s2[:], rz[j][:], s1[:], op=OP.mult)
                        s3 = sb.tile([128, N], FP32, tag="s3", bufs=2, name=f"s3{g}_{t}_{j}")
                        nc.vector.tensor_tensor(s3[:], s2[:], pgi[:], op=OP.add)
                        nn = sb.tile([128, N], FP32, tag="nn", bufs=2, name=f"nn{g}_{t}_{j}")
                        nc.scalar.activation(nn[:], s3[:], AT.Tanh, bias=bihn_sb[j][:])
                        d1 = sb.tile([128, N], FP32, tag="d1", bufs=2, name=f"d1{g}_{t}_{j}")
                        nc.vector.tensor_tensor(d1[:], H_cur[j][:], nn[:], op=OP.subtract)
                        zd = sb.tile([128, N], FP32, tag="zd", bufs=2, name=f"zd{g}_{t}_{j}")
                        nc.vector.tensor_tensor(zd[:], rz[2 + j][:], d1[:], op=OP.mult)
                        hp = sb.tile([128, N], FP32, tag="hp", bufs=2, name=f"hp{g}_{t}_{j}")
                        nc.vector.tensor_tensor(hp[:], nn[:], zd[:], op=OP.add)
                        d2 = sb.tile([128, N], FP32, tag="d2", bufs=2, name=f"d2{g}_{t}_{j}")
                        nc.vector.tensor_tensor(d2[:], hp[:], H_cur[j][:], op=OP.subtract)
                        md = sb.tile([128, N], FP32, tag="md", bufs=2, name=f"md{g}_{t}_{j}")
                        nc.vector.tensor_tensor(md[:], mv_sb[:], d2[:], op=OP.mult)
                        hnw = sb.tile([128, N], F32R, tag=f"H{g}_{j}", bufs=2,
                                      name=f"H{g}_{t + 1}_{j}")
                        nc.vector.tensor_tensor(hnw[:], H_cur[j][:], md[:], op=OP.add)
                        H_new.append(hnw)
                    H_cur = H_new

                H_final[g] = H_cur
                if not last:
                    for c_ in range(4, NCH):
                        states[g + 1]["ph1a"](c_)
                    for c_ in range(NCH):
                        states[g + 1]["ph1b"](c_)

            # ---- pair gather (both graphs) ----
            for g in range(BG):
                H_cur = H_final[g]
                h_nm = sb.tile([N, M], F32R, tag="h_nm", bufs=2, name=f"hnm_t{g}")
                for j in range(KM):
                    pt = ps_sm.tile([N, 128], FP32, tag="sm", name=f"htr{g}_{j}")
                    nc.tensor.transpose(pt[:], H_cur[j][:].bitcast(FP32), id_sb[:, :])
                    nc.vector.tensor_copy(h_nm[:, j * 128:(j + 1) * 128], pt[:])
                gt = sb.tile([N, 128], F32R, tag="gt", bufs=2, name=f"gt{g}")
                nc.sync.dma_start(gt[:], gmat[g])
                for j in range(KM):
                    pci = ps_mm.tile([128, 128], FP32, tag="mm", name=f"ci{g}_{j}")
                    nc.tensor.matmul(pci[:], h_nm[:, j * 128:(j + 1) * 128], gt[:], **MMOP)
                    nc.vector.tensor_copy(ci_sb[j][:, g * 128:(g + 1) * 128], pci[:])

            # ================= phase 4: classifiers (batched) =================
            for key in ("lr", "cr", "mr"):
                od = cls_od[key]
                h1 = []
                for j in range(KH):
                    ph = ps_mm.tile([128, BG * 128], FP32, tag="mm", name=f"c1{key}_{j}")
                    for k in range(KM):
                        nc.tensor.matmul(ph[:], c1_sb[key][k][:, j * 128:(j + 1) * 128],
                                         ci_sb[k][:], start=(k == 0), stop=(k == KM - 1))
                    ht = sb.tile([128, BG * 128], F32R, tag=f"h1{key}", bufs=2,
                                 name=f"h1{key}_{j}")
                    nc.scalar.activation(ht[:], ph[:], AT.Relu, bias=cb1_sb[key][j][:])
                    h1.append(ht)
                po = ps_mm.tile([od, BG * 128], FP32, tag="mm", name=f"c2{key}")
                for k in range(KH):
                    nc.tensor.matmul(po[:], c2_sb[key][k][:], h1[k][:],
                                     start=(k == 0), stop=(k == KH - 1))
                osb = sb.tile([od, BG * 128], FP32, tag="osb", bufs=2, name=f"osb{key}")
                nc.scalar.activation(osb[:], po[:], AT.Identity, bias=cb2_sb[key][:])
                for g in range(BG):
                    ptr = ps_sm.tile([128, od], FP32, tag="sm", name=f"otr{key}_{g}")
                    nc.tensor.transpose(ptr[:], osb[:, g * 128:(g + 1) * 128],
                                        id_sb[0:od, 0:od])
                    og = sb.tile([128, od], FP32, tag="og", bufs=2, name=f"og{key}_{g}")
                    nc.vector.tensor_copy(og[:], ptr[:])
                    nc.sync.dma_start(outmap[key][g], og[:])

    nc.compile()
    return nc


def _get_runner():
    if "runner" in _CACHE:
        return _CACHE["runner"]
    import jax
    import numpy as np
    from jax.experimental.shard_map import shard_map
    from jax.sharding import Mesh, NamedSharding, PartitionSpec
    from concourse import mybir
    from concourse.bass2jax import (_bass_exec_p, install_neuronx_cc_hook,
                                    partition_id_tensor)

    nc = _build_program()
    _CACHE["nc"] = nc
    install_neuronx_cc_hook()

    pname = nc.partition_id_tensor.name if nc.partition_id_tensor else None
    in_names, out_names, out_avals, zero_outs = [], [], [], []
    for alloc in nc.m.functions[0].allocations:
        if not isinstance(alloc, mybir.MemoryLocationSet):
            continue
        name = alloc.memorylocations[0].name
        if alloc.kind == "ExternalInput":
            if name != pname:
                in_names.append(name)
        elif alloc.kind == "ExternalOutput":
            out_names.append(name)
            shape = tuple(alloc.tensor_shape)
            dtype = mybir.dt.np(alloc.dtype)
            out_avals.append(jax.core.ShapedArray(shape, dtype))
            zero_outs.append(np.zeros(shape, dtype))
    n_params = len(in_names)
    all_in_names = in_names + out_names
    if pname is not None:
        all_in_names = all_in_names + [pname]

    def _body(*args):
        operands = list(args)
        if pname is not None:
            operands.append(partition_id_tensor())
        outs = _bass_exec_p.bind(
            *operands,
            out_avals=tuple(out_avals),
            in_names=tuple(all_in_names),
            out_names=tuple(out_names),
            lowering_input_output_aliases=(),
            sim_require_finite=False,
            sim_require_nnan=False,
            nc=nc,
        )
        return tuple(outs)

    devices = jax.devices()[:NCORES]
    mesh = Mesh(np.asarray(devices), ("core",))
    n_all = n_params + len(zero_outs)
    sharded = jax.jit(
        shard_map(_body, mesh=mesh,
                  in_specs=(PartitionSpec("core"),) * n_all,
                  out_specs=(PartitionSpec("core"),) * len(out_names),
                  check_rep=False),
        keep_unused=True,
    )
    sharding = NamedSharding(mesh, PartitionSpec("core"))
    runner = dict(sharded=sharded, in_names=in_names, out_names=out_names,
                  out_avals=out_avals, zero_outs=zero_outs, sharding=sharding,
                  mesh=mesh)
    _CACHE["runner"] = runner
    return runner


def _pack_biases(inputs):
    f32 = np.float32
    cols = []

    def add(vec, chunks=None):
        v = np.asarray(vec, f32).ravel()
        n = (len(v) + 127) // 128 if chunks is None else chunks
        for k in range(n):
            c = np.zeros(128, f32)
            seg = v[k * 128:(k + 1) * 128]
            c[:len(seg)] = seg
            cols.append(c)

    add(inputs["b_edge_rs"])                      # 2
    add(inputs["b_node_rs"])                      # 2
    add(inputs["link_b1"])                        # 4
    add(inputs["link_b2"])                        # 4
    add(inputs["link_b3"])                        # 1
    add(inputs["bm"])                             # 2
    add(np.asarray(inputs["bih"], f32)[0:2 * M]
        + np.asarray(inputs["bhh"], f32)[0:2 * M])  # 4
    add(np.asarray(inputs["bih"], f32)[2 * M:])   # 2
    add(np.asarray(inputs["bhh"], f32)[2 * M:])   # 2
    for k in ("lr", "cr", "mr"):
        add(inputs[f"{k}_b1"])                    # 4 each
    for k in ("lr", "cr", "mr"):
        add(inputs[f"{k}_b2"], chunks=1)          # 1 each
    out = np.stack(cols, axis=1)
    assert out.shape[1] == NBIAS, out.shape
    return np.ascontiguousarray(out)


def _preprocess(inputs):
    """Host-side prep: per-core input dict values, each shaped [ncores*d0, ...]."""
    f32 = np.float32
    rsf = np.ascontiguousarray(inputs["relative_spatial_feature"], f32)
    cnf = np.ascontiguousarray(inputs["concatenated_node_features"], f32)
    num_obj = np.asarray(inputs["num_obj"])
    pairs = np.asarray(inputs["object_pairs"])

    T = lambda a: np.ascontiguousarray(np.asarray(a, f32).T)
    col = lambda a: np.ascontiguousarray(np.asarray(a, f32).reshape(-1, 1))

    per_graph_hnmask = np.zeros((B, 128, N), f32)
    per_graph_maskv = np.zeros((B, 128, N), f32)
    per_graph_mask2 = np.zeros((B, NE), f32)
    per_graph_G = np.zeros((B, N, 128), f32)
    for b in range(B):
        valid = (np.arange(N) < int(num_obj[b]))
        per_graph_hnmask[b, :, :] = np.where(valid, 0.0, NEG)[None, :]
        per_graph_maskv[b, :, :] = valid.astype(f32)[None, :]
        per_graph_mask2[b, :] = (valid[:, None] & valid[None, :]).astype(f32).ravel()
        gm = np.zeros((N, 128), f32)
        for p in range(128):
            gm[int(pairs[b, p, 0]), p] += 0.5
            gm[int(pairs[b, p, 1]), p] += 0.5
        per_graph_G[b] = gm

    shared = {
        "wer_t": T(inputs["W_edge_rs"]), "wnr_t": T(inputs["W_node_rs"]),
        "w1_t": T(inputs["link_W1"]), "w2_t": T(inputs["link_W2"]),
        "w3_t": T(inputs["link_W3"]),
        "wmn_t": T(inputs["Wm_node"]), "wme_t": T(inputs["Wm_edge"]),
        "wih_t": T(inputs["Wih"]), "whh_t": T(inputs["Whh"]),
        "lr1_t": T(inputs["lr_W1"]), "lr2_t": T(inputs["lr_W2"]),
        "cr1_t": T(inputs["cr_W1"]), "cr2_t": T(inputs["cr_W2"]),
        "mr1_t": T(inputs["mr_W1"]), "mr2_t": T(inputs["mr_W2"]),
        "ones": np.ones((1, 128), f32), "ident": np.eye(128, dtype=f32),
        "bias_all": _pack_biases(inputs),
    }

    concat = {}
    concat["rsf"] = rsf.reshape(B, NE, EF)          # [16*2? -> (8*2, NE, EF)]
    concat["cnf"] = cnf
    concat["hnmask"] = per_graph_hnmask
    concat["maskv"] = per_graph_maskv
    concat["mask2"] = per_graph_mask2
    concat["gmat"] = per_graph_G
    for k, v in shared.items():
        concat[k] = np.concatenate([v] * NCORES, axis=0)
    return concat


def _postprocess(out_map):
    lr = out_map["out_lr"].reshape(B, 128, 4)
    cr = out_map["out_cr"].reshape(B, 128, 6)
    mr = out_map["out_mr"].reshape(B, 128, 17)
    pa = out_map["out_pa"].reshape(B, N, N)
    return lr, cr, mr, pa


def _run_concat(concat):
    import jax
    r = _get_runner()
    args = [np.ascontiguousarray(concat[n]) for n in r["in_names"]]
    zeros = [np.zeros((NCORES * z.shape[0], *z.shape[1:]), z.dtype)
             for z in r["zero_outs"]]
    outs = r["sharded"](*args, *zeros)
    return {n: np.asarray(outs[i]) for i, n in enumerate(r["out_names"])}


def kernel(**inputs):
    concat = _preprocess(inputs)
    out_map = _run_concat(concat)
    return _postprocess(out_map)


if __name__ == "__main__":
    rng = np.random.default_rng(0)
    print("building...")
    _get_runner()
    print("built ok")


# revision 58
# speedup vs baseline: 1.0396x; 1.0396x over previous
"""GPNN message-passing kernel for 8x Trainium2 NeuronCores.

Strategy:
  - Pure data parallel over batch: B=16 graphs -> 2 graphs per core.
  - Feature-major layout on chip: activations stored [feat_partitions, edges].
  - float32r matmuls (full PE rate at N>=512, ~2e-4 rel err).
  - Edge-state parity trick: E_{t+1}[w,v] = msg_t[v,w] is stored in place
    (no physical transpose); even/odd iterations flip the interpretation of
    the two N axes.  Invalid-pair entries of E are dead values (masked out of
    every output path), so they may hold garbage.
  - w-validity masking folded into hn via -1e30 additive mask before relu.
  - gate broadcast across partitions via a K=1 ones-matmul into PSUM,
    fused relu+gate-multiply via scalar_tensor_tensor.
"""

import os
import sys

for _p in ("/opt/trn_rl_repo",):
    if _p not in sys.path:
        sys.path.insert(0, _p)

import numpy as np

B, N, NF, EF, M, HID = 16, 64, 1024, 256, 256, 512
L_PROP = 3
NCORES = 8
BG = B // NCORES          # graphs per core = 2
NE = N * N                # 4096 edges per graph
ECH = 512                 # edges per chunk
NCH = NE // ECH           # 8 chunks
VCH = ECH // N            # 8 outer-axis rows per chunk
NEG = -1.0e30
A1_WINDOW = 2             # chunks of a1 emitted ahead
NBIAS = 2 + 4 + 2 + 4 + 4 + 1 + 2 + 4 + 2 + 2 + 12 + 3   # packed bias cols = 42

_CACHE = {}


def _build_program():
    import concourse.bass as bass
    import concourse.tile as tile
    from concourse import bacc, mybir

    F32R = mybir.dt.float32r
    FP32 = mybir.dt.float32
    AT = mybir.ActivationFunctionType
    OP = mybir.AluOpType
    AX = mybir.AxisListType

    nc = bacc.Bacc("TRN2", target_bir_lowering=False, debug=False,
                   num_devices=NCORES)

    def din(name, shape, dt=FP32):
        return nc.dram_tensor(name, list(shape), dt, kind="ExternalInput").ap()

    def dout(name, shape, dt=FP32):
        return nc.dram_tensor(name, list(shape), dt, kind="ExternalOutput").ap()

    # ---- inputs ----
    rsf = din("rsf", [BG, NE, EF])               # relative_spatial_feature
    cnf = din("cnf", [BG, N, NF])                # concatenated_node_features
    # weights (pre-transposed host-side to [K, M] layout), f32r
    wen_t = din("wen_t", [EF, M], F32R)
    w1e_t = din("w1e_t", [EF, HID], F32R)
    wnr_t = din("wnr_t", [NF, M], F32R)
    w1_t = din("w1_t", [M, HID], F32R)
    w2_t = din("w2_t", [HID, HID], F32R)
    w3_t = din("w3_t", [HID, 1], F32R)
    wmn_t = din("wmn_t", [M, M], F32R)
    wih_t = din("wih_t", [M, 3 * M], F32R)
    whh_t = din("whh_t", [M, 3 * M], F32R)
    cls_w1 = {k: din(f"{k}1_t", [M, HID], F32R) for k in ("lr", "cr", "mr")}
    cls_od = {"lr": 4, "cr": 6, "mr": 17}
    cls_w2 = {k: din(f"{k}2_t", [HID, cls_od[k]], F32R) for k in cls_od}
    gmat = din("gmat", [BG, N, 128], F32R)       # pair-gather matrix
    ones = din("ones", [1, 128], F32R)
    ident = din("ident", [128, 128])
    # all biases packed column-wise: [128, NBIAS]
    bias_all = din("bias_all", [128, NBIAS])
    # masks
    hnmask = din("hnmask", [BG, 128, N])         # 0 / -1e30 along w
    maskv = din("maskv", [BG, 128, N])           # 1.0 / 0.0 along v
    mask2 = din("mask2", [BG, NE])               # valid pair mask flat

    # ---- outputs ----
    out_lr = dout("out_lr", [BG, 128, 4])
    out_cr = dout("out_cr", [BG, 128, 6])
    out_mr = dout("out_mr", [BG, 128, 17])
    out_pa = dout("out_pa", [BG, NE])

    KM, KH, KNF = M // 128, HID // 128, NF // 128  # 2, 4, 8

    with tile.TileContext(nc, trace_sim=bool(os.environ.get("KTRACE"))) as tc:
        from contextlib import ExitStack
        ctx = ExitStack()
        with ctx:
            wp = ctx.enter_context(tc.tile_pool(name="wp", bufs=1))
            sb = ctx.enter_context(tc.tile_pool(name="sb", bufs=1))
            ps_mm = ctx.enter_context(tc.tile_pool(name="ps_mm", bufs=4, space="PSUM"))
            ps_pd = ctx.enter_context(tc.tile_pool(name="ps_pd", bufs=1, space="PSUM"))
            ps_sm = ctx.enter_context(tc.tile_pool(name="ps_sm", bufs=2, space="PSUM"))

            _wq = [nc.sync, nc.scalar]
            _wqi = [0]

            def wtile(src, k, width, dt=F32R, pfx="w"):
                t = wp.tile([128, width], dt, name=f"{pfx}_{src.name}_{k}")
                _wq[_wqi[0] % 2].dma_start(t[:], src[k * 128:(k + 1) * 128, :])
                _wqi[0] += 1
                return t

            bias_sb = wp.tile([128, NBIAS], FP32, name="bias_sb")
            nc.sync.dma_start(bias_sb[:], bias_all)
            nf_pre = {}
            nf_pre[0] = sb.tile([N, NF], FP32, tag="nf_nm", bufs=1, name="nfnm_pre0")
            nc.sync.dma_start(nf_pre[0][:], cnf[0])
            _bcol = iter(range(NBIAS))
            def bslice(rows=128):
                i = next(_bcol)
                return bias_sb[0:rows, i:i + 1]
            ben_sb = [bslice() for k in range(KM)]
            b1t0_sb = [bslice() for k in range(KH)]
            bnode_sb = [bslice() for k in range(KM)]
            b1_sb = [bslice() for k in range(KH)]
            b2_sb = [bslice() for k in range(KH)]
            b3_sb = bslice(1)
            bm_sb = [bslice() for k in range(KM)]
            brz_sb = [bslice() for k in range(2 * M // 128)]
            bihn_sb = [bslice() for k in range(KM)]
            bhhn_sb = [bslice() for k in range(KM)]
            cb1_sb = {k: [bslice() for j in range(KH)] for k in cls_od}
            cb2_sb = {k: bslice(cls_od[k]) for k in cls_od}
            # ---- load weights (ordered by first use, spread over 2 rings) ----
            id_sb = wp.tile([128, 128], FP32, name="id_sb")
            nc.scalar.dma_start(id_sb[:], ident)
            wen_sb = [wtile(wen_t, k, M) for k in range(KM)]
            w1e_sb = [wtile(w1e_t, k, HID) for k in range(KM)]
            wnr_sb = [wtile(wnr_t, k, M) for k in range(KNF)]
            w1_sb = [wtile(w1_t, k, HID) for k in range(KM)]
            w2_sb = [wtile(w2_t, k, HID) for k in range(KH)]
            w3_sb = [wtile(w3_t, k, 1) for k in range(KH)]
            wmn_sb = [wtile(wmn_t, k, M) for k in range(KM)]
            ones_sb = wp.tile([1, 128], F32R, name="ones_sb")
            nc.scalar.dma_start(ones_sb[:], ones)



            MMOP = dict(start=True, stop=True)
            outmap = {"lr": out_lr, "cr": out_cr, "mr": out_mr}
            H_final = [None] * BG
            ci_sb = [sb.tile([128, BG * 128], F32R, name=f"ci_{k}") for k in range(KM)]

            def make_graph_state(g):
                """Emit phase 0 (node features, H0, masks) and return per-graph
                state with phase-1 emitters."""
                st = {}
                if g in nf_pre:
                    nf_nm = nf_pre[g]
                else:
                    nf_nm = sb.tile([N, NF], FP32, tag="nf_nm", bufs=1, name=f"nfnm{g}")
                    nc.sync.dma_start(nf_nm[:], cnf[g])
                nf_fm = []
                for k8 in range(KNF):
                    pt = ps_mm.tile([128, N], FP32, tag="mm", name=f"nft{g}_{k8}")
                    nc.tensor.transpose(pt[:], nf_nm[:, k8 * 128:(k8 + 1) * 128],
                                        id_sb[0:N, 0:N])
                    t_ = sb.tile([128, N], F32R, tag="nf_fm", bufs=8,
                                 name=f"nffm{g}_{k8}")
                    nc.vector.tensor_copy(t_[:], pt[:])
                    nf_fm.append(t_)
                H_cur = []
                for j in range(KM):
                    ph = ps_mm.tile([128, N], FP32, tag="mm", name=f"h0p{g}_{j}")
                    for k in range(KNF):
                        nc.tensor.matmul(ph[:], wnr_sb[k][:, j * 128:(j + 1) * 128],
                                         nf_fm[k][:], start=(k == 0), stop=(k == KNF - 1))
                    ht = sb.tile([128, N], F32R, tag=f"H{g}_{j}", bufs=2,
                                 name=f"H0_{g}_{j}")
                    nc.scalar.activation(ht[:], ph[:], AT.Identity, bias=bnode_sb[j][:])
                    H_cur.append(ht)
                st["H"] = H_cur
                hnm_sb = sb.tile([128, N], FP32, tag="hnm", bufs=2, name=f"hnm{g}")
                nc.sync.dma_start(hnm_sb[:], hnmask[g])
                mv_sb = sb.tile([128, N], FP32, tag="mv", bufs=2, name=f"mv{g}")
                nc.sync.dma_start(mv_sb[:], maskv[g])
                st["hnm"], st["mv"] = hnm_sb, mv_sb

                E_t = [[None] * NCH for _ in range(KM)]
                en_big = [sb.tile([128, NE], FP32, tag=f"en{j}", name=f"en{g}_{j}")
                          for j in range(KM)]
                st["E"], st["en"] = E_t, en_big
                a1_pre = [None] * NCH
                st["a1_pre"] = a1_pre

                def emit_a1g(c, sfx, wsrc=None, bsrc=None):
                    wsrc = w1_sb if wsrc is None else wsrc
                    bsrc = b1_sb if bsrc is None else bsrc
                    tiles = []
                    for j in range(KH):
                        pa = ps_mm.tile([128, ECH], FP32, tag="mm",
                                        name=f"a1p{g}_{sfx}_{c}_{j}")
                        for k in range(KM):
                            nc.tensor.matmul(pa[:], wsrc[k][:, j * 128:(j + 1) * 128],
                                             E_t[k][c][:], start=(k == 0),
                                             stop=(k == KM - 1))
                        at = sb.tile([128, ECH], F32R, tag="a1", bufs=4 * A1_WINDOW + 2,
                                     name=f"a1{g}_{sfx}_{c}_{j}")
                        nc.scalar.activation(at[:], pa[:], AT.Relu, bias=bsrc[j][:])
                        tiles.append(at)
                    return tiles
                st["emit_a1g"] = emit_a1g

                def ph1a(c, interleaved=False):
                    tp_pool, tp_tag = (ps_sm, "sm") if interleaved else (ps_mm, "mm")
                    for j in range(KM):
                        E_t[j][c] = sb.tile([128, ECH], F32R, tag=f"E{j}_{c}",
                                            name=f"E{g}_{j}_{c}")
                    rnm = sb.tile([128, 4 * EF], FP32, tag="rnm", bufs=2,
                                  name=f"rnm{g}_{c}")
                    nc.sync.dma_start(
                        rnm[:].rearrange("p (et f) -> p et f", et=4),
                        rsf[g, c * ECH:(c + 1) * ECH, :].rearrange(
                            "(et p) f -> p et f", et=4))
                    for et in range(ECH // 128):
                        for j in range(KM):
                            pt = tp_pool.tile([128, 128], FP32, tag=tp_tag,
                                              name=f"rt{g}_{c}_{et}_{j}")
                            o0 = et * EF + j * 128
                            nc.tensor.transpose(pt[:], rnm[:, o0:o0 + 128],
                                                id_sb[:, :])
                            if (et + j) % 2 == 0:
                                nc.vector.tensor_copy(
                                    E_t[j][c][:, et * 128:(et + 1) * 128], pt[:])
                            else:
                                nc.scalar.copy(
                                    E_t[j][c][:, et * 128:(et + 1) * 128], pt[:])
                    if c < A1_WINDOW:
                        a1_pre[c] = emit_a1g(c, "p", wsrc=w1e_sb, bsrc=b1t0_sb)
                st["ph1a"] = ph1a

                def ph1b(c):
                    for j in range(KM):
                        pen = ps_mm.tile([128, ECH], FP32, tag="mm",
                                         name=f"enp{g}_{c}_{j}")
                        for k in range(KM):
                            nc.tensor.matmul(pen[:], wen_sb[k][:, j * 128:(j + 1) * 128],
                                             E_t[k][c][:], start=(k == 0),
                                             stop=(k == KM - 1))
                        nc.vector.tensor_scalar(en_big[j][:, c * ECH:(c + 1) * ECH],
                                                pen[:], ben_sb[j][:], None, op0=OP.add)
                st["ph1b"] = ph1b
                return st

            states = [None] * BG
            states[0] = make_graph_state(0)
            for c in range(NCH):
                states[0]["ph1a"](c)
                states[0]["ph1b"](c)

            # late-needed weights: load during phase 1 compute
            wih_sb = [wtile(wih_t, k, 3 * M) for k in range(KM)]
            whh_sb = [wtile(whh_t, k, 3 * M) for k in range(KM)]
            c1_sb = {k: [wtile(cls_w1[k], j, HID) for j in range(KM)] for k in cls_od}
            c2_sb = {k: [wtile(cls_w2[k], j, cls_od[k]) for j in range(KH)]
                     for k in cls_od}

            for g in range(BG):
                st = states[g]
                E_t, en_big = st["E"], st["en"]
                H_cur, hnm_sb, mv_sb = st["H"], st["hnm"], st["mv"]
                a1_pre, emit_a1g = st["a1_pre"], st["emit_a1g"]
                last = (g == BG - 1)

                for t in range(L_PROP):
                    even = (t % 2 == 0)
                    final_t = (t == L_PROP - 1)
                    hn_m = []
                    for j in range(KM):
                        phn = ps_sm.tile([128, N], FP32, tag="sm", name=f"hnp{g}_{t}_{j}")
                        for k in range(KM):
                            nc.tensor.matmul(phn[:], wmn_sb[k][:, j * 128:(j + 1) * 128],
                                             H_cur[k][:], start=(k == 0), stop=(k == KM - 1))
                        hnt = sb.tile([128, N], FP32, tag="hn", bufs=4,
                                      name=f"hn{g}_{t}_{j}")
                        nc.scalar.activation(hnt[:], phn[:], AT.Identity, bias=bm_sb[j][:])
                        hm = sb.tile([128, N], FP32, tag="hnm2", bufs=4,
                                     name=f"hnm{g}_{t}_{j}")
                        nc.vector.tensor_tensor(hm[:], hnt[:], hnm_sb[:], op=OP.add)
                        hn_m.append(hm)

                    a1_t = [None] * NCH
                    m_fin = [sb.tile([128, N], F32R, tag=f"ms{j}", bufs=2,
                                     name=f"msum{g}_{t}_{j}") for j in range(KM)]
                    mps = None
                    if not even:
                        mps = [sb.tile([128, NCH * N], FP32, tag=f"mps{j}", bufs=1,
                                       name=f"mps{g}_{t}_{j}") for j in range(KM)]
                    if t == 0:
                        for c0 in range(A1_WINDOW):
                            a1_t[c0] = a1_pre[c0]
                    else:
                        for c0 in range(A1_WINDOW):
                            a1_t[c0] = prefetched_a1[c0]

                    prefetched_next = [None] * NCH
                    for c in range(NCH):
                        if c + A1_WINDOW < NCH:
                            if t == 0:
                                a1_t[c + A1_WINDOW] = emit_a1g(
                                    c + A1_WINDOW, "t0", wsrc=w1e_sb, bsrc=b1t0_sb)
                            else:
                                a1_t[c + A1_WINDOW] = emit_a1g(c + A1_WINDOW, f"t{t}")
                        # a2
                        a2_t = []
                        for j in range(KH):
                            pa = ps_mm.tile([128, ECH], FP32, tag="mm",
                                            name=f"a2p{g}_{t}_{c}_{j}")
                            for k in range(KH):
                                nc.tensor.matmul(pa[:], w2_sb[k][:, j * 128:(j + 1) * 128],
                                                 a1_t[c][k][:], start=(k == 0), stop=(k == KH - 1))
                            at = sb.tile([128, ECH], F32R, tag="a2", bufs=4,
                                         name=f"a2{g}_{t}_{c}_{j}")
                            if j < 2:
                                nc.scalar.activation(at[:], pa[:], AT.Relu, bias=b2_sb[j][:])
                            else:
                                nc.vector.tensor_scalar(at[:], pa[:], b2_sb[j][:], 0.0,
                                                        op0=OP.add, op1=OP.max)
                            a2_t.append(at)
                        a1_t[c] = None
                        # padj
                        ppd = ps_pd.tile([1, ECH], FP32, tag="pd", name=f"pdp{g}_{t}_{c}")
                        for k in range(KH):
                            nc.tensor.matmul(ppd[:], w3_sb[k][:], a2_t[k][:],
                                             start=(k == 0), stop=(k == KH - 1))
                        gate = sb.tile([1, ECH], F32R, tag="gate", bufs=2,
                                       name=f"gate{g}_{t}_{c}")
                        nc.scalar.activation(gate[:], ppd[:], AT.Sigmoid, bias=b3_sb[:])
                        if final_t:
                            m2t = sb.tile([1, ECH], FP32, tag="m2", bufs=1,
                                          name=f"m2{g}_{c}")
                            nc.sync.dma_start(
                                m2t[:], mask2[g, c * ECH:(c + 1) * ECH].unsqueeze(0))
                            pam = sb.tile([1, ECH], FP32, tag="pam", bufs=1,
                                          name=f"pam{g}_{c}")
                            nc.vector.scalar_tensor_tensor(
                                pam[:], ppd[:], b3_sb[:], m2t[:],
                                op0=OP.add, op1=OP.mult)
                            nc.sync.dma_start(
                                out_pa[g, c * ECH:(c + 1) * ECH].unsqueeze(0), pam[:])
                        pgb = ps_sm.tile([128, ECH], FP32, tag="gb", bufs=1,
                                         name=f"gb{g}_{t}_{c}")
                        nc.tensor.matmul(pgb[:], ones_sb[:], gate[:], **MMOP)
                        for j in range(KM):
                            tmp = sb.tile([128, ECH], FP32, tag="tmp", bufs=2,
                                          name=f"tmp{g}_{t}_{c}_{j}")
                            tmp3 = tmp[:].rearrange("p (a b) -> p a b", a=VCH)
                            if even:
                                en_v = en_big[j][:, c * ECH:(c + 1) * ECH].rearrange(
                                    "p (a b) -> p a b", a=VCH)
                                hn_v = hn_m[j][:].unsqueeze(1).broadcast_to((128, VCH, N))
                            else:
                                en_v = en_big[j][:].rearrange(
                                    "p (v w) -> p v w", v=N).transpose([0, 2, 1])[
                                    :, c * VCH:(c + 1) * VCH, :]
                                hn_v = hn_m[j][:, c * VCH:(c + 1) * VCH].unsqueeze(
                                    2).broadcast_to((128, VCH, N))
                            nc.vector.tensor_tensor(tmp3, en_v, hn_v, op=OP.add)
                            mdst = tmp if final_t else E_t[j][c]
                            nc.vector.scalar_tensor_tensor(
                                mdst[:], tmp[:], 0.0, pgb[:],
                                op0=OP.max, op1=OP.mult)
                            with nc.allow_low_precision(reason="f32r msum"):
                                if even:
                                    nc.vector.tensor_reduce(
                                        m_fin[j][:, c * VCH:(c + 1) * VCH],
                                        mdst[:].rearrange("p (a b) -> p a b", a=VCH),
                                        axis=AX.X, op=OP.add)
                                else:
                                    nc.vector.tensor_reduce(
                                        mps[j][:, c * N:(c + 1) * N],
                                        mdst[:].rearrange("p (a b) -> p b a", a=VCH),
                                        axis=AX.X, op=OP.add)

                    if t + 1 < L_PROP:
                        for c0 in range(A1_WINDOW):
                            prefetched_next[c0] = emit_a1g(c0, f"t{t + 1}n")
                        prefetched_a1 = prefetched_next
                    if final_t and not last:
                        states[g + 1] = make_graph_state(g + 1)
                        for c_ in range(4):
                            states[g + 1]["ph1a"](c_)
                    if not even:
                        with nc.allow_low_precision(reason="f32r msum"):
                            for j in range(KM):
                                nc.vector.tensor_reduce(
                                    m_fin[j][:],
                                    mps[j][:].rearrange("p (c b) -> p b c", c=NCH),
                                    axis=AX.X, op=OP.add)
                    # ---- GRU ----
                    rz = []
                    for j in range(4):
                        prz = ps_sm.tile([128, N], FP32, tag="sm", name=f"rz{g}_{t}_{j}")
                        for k in range(KM):
                            nc.tensor.matmul(prz[:], wih_sb[k][:, j * 128:(j + 1) * 128],
                                             m_fin[k][:], start=(k == 0), stop=False)
                        for k in range(KM):
                            nc.tensor.matmul(prz[:], whh_sb[k][:, j * 128:(j + 1) * 128],
                                             H_cur[k][:], start=False, stop=(k == KM - 1))
                        rzt = sb.tile([128, N], FP32, tag="rz", bufs=6, name=f"rzt{g}_{t}_{j}")
                        nc.scalar.activation(rzt[:], prz[:], AT.Sigmoid, bias=brz_sb[j][:])
                        rz.append(rzt)
                    H_new = []
                    for j in range(KM):
                        jj = 4 + j
                        pgi = ps_sm.tile([128, N], FP32, tag="sm", name=f"gin{g}_{t}_{j}")
                        for k in range(KM):
                            nc.tensor.matmul(pgi[:], wih_sb[k][:, jj * 128:(jj + 1) * 128],
                                             m_fin[k][:], start=(k == 0), stop=(k == KM - 1))
                        pgh = ps_sm.tile([128, N], FP32, tag="sm", name=f"ghn{g}_{t}_{j}")
                        for k in range(KM):
                            nc.tensor.matmul(pgh[:], whh_sb[k][:, jj * 128:(jj + 1) * 128],
                                             H_cur[k][:], start=(k == 0), stop=(k == KM - 1))
                        s1 = sb.tile([128, N], FP32, tag="s1", bufs=2, name=f"s1{g}_{t}_{j}")
                        nc.scalar.activation(s1[:], pgh[:], AT.Identity, bias=bhhn_sb[j][:])
                        s2 = sb.tile([128, N], FP32, tag="s2", bufs=2, name=f"s2{g}_{t}_{j}")
                        nc.vector.tensor_tensor(s2[:], rz[j][:], s1[:], op=OP.mult)
                        s3 = sb.tile([128, N], FP32, tag="s3", bufs=2, name=f"s3{g}_{t}_{j}")
                        nc.vector.tensor_tensor(s3[:], s2[:], pgi[:], op=OP.add)
                        nn = sb.tile([128, N], FP32, tag="nn", bufs=2, name=f"nn{g}_{t}_{j}")
                        nc.scalar.activation(nn[:], s3[:], AT.Tanh, bias=bihn_sb[j][:])
                        d1 = sb.tile([128, N], FP32, tag="d1", bufs=2, name=f"d1{g}_{t}_{j}")
                        nc.vector.tensor_tensor(d1[:], H_cur[j][:], nn[:], op=OP.subtract)
                        zd = sb.tile([128, N], FP32, tag="zd", bufs=2, name=f"zd{g}_{t}_{j}")
                        nc.vector.tensor_tensor(zd[:], rz[2 + j][:], d1[:], op=OP.mult)
                        hp = sb.tile([128, N], FP32, tag="hp", bufs=2, name=f"hp{g}_{t}_{j}")
                        nc.vector.tensor_tensor(hp[:], nn[:], zd[:], op=OP.add)
                        d2 = sb.tile([128, N], FP32, tag="d2", bufs=2, name=f"d2{g}_{t}_{j}")
                        nc.vector.tensor_tensor(d2[:], hp[:], H_cur[j][:], op=OP.subtract)
                        md = sb.tile([128, N], FP32, tag="md", bufs=2, name=f"md{g}_{t}_{j}")
                        nc.vector.tensor_tensor(md[:], mv_sb[:], d2[:], op=OP.mult)
                        hnw = sb.tile([128, N], F32R, tag=f"H{g}_{j}", bufs=2,
                                      name=f"H{g}_{t + 1}_{j}")
                        nc.vector.tensor_tensor(hnw[:], H_cur[j][:], md[:], op=OP.add)
                        H_new.append(hnw)
                    H_cur = H_new

                H_final[g] = H_cur
                if not last:
                    for c_ in range(4, NCH):
                        states[g + 1]["ph1a"](c_)
                    for c_ in range(NCH):
                        states[g + 1]["ph1b"](c_)

            # ---- pair gather (both graphs) ----
            for g in range(BG):
                H_cur = H_final[g]
                h_nm = sb.tile([N, M], F32R, tag="h_nm", bufs=2, name=f"hnm_t{g}")
                for j in range(KM):
                    pt = ps_sm.tile([N, 128], FP32, tag="sm", name=f"htr{g}_{j}")
                    nc.tensor.transpose(pt[:], H_cur[j][:].bitcast(FP32), id_sb[:, :])
                    nc.vector.tensor_copy(h_nm[:, j * 128:(j + 1) * 128], pt[:])
                gt = sb.tile([N, 128], F32R, tag="gt", bufs=2, name=f"gt{g}")
                nc.sync.dma_start(gt[:], gmat[g])
                for j in range(KM):
                    pci = ps_mm.tile([128, 128], FP32, tag="mm", name=f"ci{g}_{j}")
                    nc.tensor.matmul(pci[:], h_nm[:, j * 128:(j + 1) * 128], gt[:], **MMOP)
                    nc.vector.tensor_copy(ci_sb[j][:, g * 128:(g + 1) * 128], pci[:])

            # ================= phase 4: classifiers (batched) =================
            for key in ("lr", "cr", "mr"):
                od = cls_od[key]
                h1 = []
                for j in range(KH):
                    ph = ps_mm.tile([128, BG * 128], FP32, tag="mm", name=f"c1{key}_{j}")
                    for k in range(KM):
                        nc.tensor.matmul(ph[:], c1_sb[key][k][:, j * 128:(j + 1) * 128],
                                         ci_sb[k][:], start=(k == 0), stop=(k == KM - 1))
                    ht = sb.tile([128, BG * 128], F32R, tag=f"h1{key}", bufs=2,
                                 name=f"h1{key}_{j}")
                    nc.scalar.activation(ht[:], ph[:], AT.Relu, bias=cb1_sb[key][j][:])
                    h1.append(ht)
                po = ps_mm.tile([od, BG * 128], FP32, tag="mm", name=f"c2{key}")
                for k in range(KH):
                    nc.tensor.matmul(po[:], c2_sb[key][k][:], h1[k][:],
                                     start=(k == 0), stop=(k == KH - 1))
                osb = sb.tile([od, BG * 128], FP32, tag="osb", bufs=2, name=f"osb{key}")
                nc.scalar.activation(osb[:], po[:], AT.Identity, bias=cb2_sb[key][:])
                for g in range(BG):
                    ptr = ps_sm.tile([128, od], FP32, tag="sm", name=f"otr{key}_{g}")
                    nc.tensor.transpose(ptr[:], osb[:, g * 128:(g + 1) * 128],
                                        id_sb[0:od, 0:od])
                    og = sb.tile([128, od], FP32, tag="og", bufs=2, name=f"og{key}_{g}")
                    nc.vector.tensor_copy(og[:], ptr[:])
                    nc.sync.dma_start(outmap[key][g], og[:])

    nc.compile()
    return nc


def _get_runner():
    if "runner" in _CACHE:
        return _CACHE["runner"]
    import jax
    import numpy as np
    from jax.experimental.shard_map import shard_map
    from jax.sharding import Mesh, NamedSharding, PartitionSpec
    from concourse import mybir
    from concourse.bass2jax import (_bass_exec_p, install_neuronx_cc_hook,
                                    partition_id_tensor)

    nc = _build_program()
    _CACHE["nc"] = nc
    install_neuronx_cc_hook()

    pname = nc.partition_id_tensor.name if nc.partition_id_tensor else None
    in_names, out_names, out_avals, zero_outs = [], [], [], []
    for alloc in nc.m.functions[0].allocations:
        if not isinstance(alloc, mybir.MemoryLocationSet):
            continue
        name = alloc.memorylocations[0].name
        if alloc.kind == "ExternalInput":
            if name != pname:
                in_names.append(name)
        elif alloc.kind == "ExternalOutput":
            out_names.append(name)
            shape = tuple(alloc.tensor_shape)
            dtype = mybir.dt.np(alloc.dtype)
            out_avals.append(jax.core.ShapedArray(shape, dtype))
            zero_outs.append(np.zeros(shape, dtype))
    n_params = len(in_names)
    all_in_names = in_names + out_names
    if pname is not None:
        all_in_names = all_in_names + [pname]

    def _body(*args):
        operands = list(args)
        if pname is not None:
            operands.append(partition_id_tensor())
        outs = _bass_exec_p.bind(
            *operands,
            out_avals=tuple(out_avals),
            in_names=tuple(all_in_names),
            out_names=tuple(out_names),
            lowering_input_output_aliases=(),
            sim_require_finite=False,
            sim_require_nnan=False,
            nc=nc,
        )
        return tuple(outs)

    devices = jax.devices()[:NCORES]
    mesh = Mesh(np.asarray(devices), ("core",))
    n_all = n_params + len(zero_outs)
    sharded = jax.jit(
        shard_map(_body, mesh=mesh,
                  in_specs=(PartitionSpec("core"),) * n_all,
                  out_specs=(PartitionSpec("core"),) * len(out_names),
                  check_rep=False),
        keep_unused=True,
    )
    sharding = NamedSharding(mesh, PartitionSpec("core"))
    runner = dict(sharded=sharded, in_names=in_names, out_names=out_names,
                  out_avals=out_avals, zero_outs=zero_outs, sharding=sharding,
                  mesh=mesh)
    _CACHE["runner"] = runner
    return runner


def _pack_biases(inputs):
    f32 = np.float32
    cols = []

    def add(vec, chunks=None):
        v = np.asarray(vec, f32).ravel()
        n = (len(v) + 127) // 128 if chunks is None else chunks
        for k in range(n):
            c = np.zeros(128, f32)
            seg = v[k * 128:(k + 1) * 128]
            c[:len(seg)] = seg
            cols.append(c)

    be = np.asarray(inputs["b_edge_rs"], np.float64)
    add(np.asarray(inputs["Wm_edge"], np.float64) @ be)          # b_en: 2
    add(np.asarray(inputs["link_W1"], np.float64) @ be
        + np.asarray(inputs["link_b1"], np.float64))             # b1t0: 4
    add(inputs["b_node_rs"])                      # 2
    add(inputs["link_b1"])                        # 4
    add(inputs["link_b2"])                        # 4
    add(inputs["link_b3"])                        # 1
    add(inputs["bm"])                             # 2
    add(np.asarray(inputs["bih"], f32)[0:2 * M]
        + np.asarray(inputs["bhh"], f32)[0:2 * M])  # 4
    add(np.asarray(inputs["bih"], f32)[2 * M:])   # 2
    add(np.asarray(inputs["bhh"], f32)[2 * M:])   # 2
    for k in ("lr", "cr", "mr"):
        add(inputs[f"{k}_b1"])                    # 4 each
    for k in ("lr", "cr", "mr"):
        add(inputs[f"{k}_b2"], chunks=1)          # 1 each
    out = np.stack(cols, axis=1)
    assert out.shape[1] == NBIAS, out.shape
    return np.ascontiguousarray(out)


def _preprocess(inputs):
    """Host-side prep: per-core input dict values, each shaped [ncores*d0, ...]."""
    f32 = np.float32
    rsf = np.ascontiguousarray(inputs["relative_spatial_feature"], f32)
    cnf = np.ascontiguousarray(inputs["concatenated_node_features"], f32)
    num_obj = np.asarray(inputs["num_obj"])
    pairs = np.asarray(inputs["object_pairs"])

    T = lambda a: np.ascontiguousarray(np.asarray(a, f32).T)
    col = lambda a: np.ascontiguousarray(np.asarray(a, f32).reshape(-1, 1))

    per_graph_hnmask = np.zeros((B, 128, N), f32)
    per_graph_maskv = np.zeros((B, 128, N), f32)
    per_graph_mask2 = np.zeros((B, NE), f32)
    per_graph_G = np.zeros((B, N, 128), f32)
    for b in range(B):
        valid = (np.arange(N) < int(num_obj[b]))
        per_graph_hnmask[b, :, :] = np.where(valid, 0.0, NEG)[None, :]
        per_graph_maskv[b, :, :] = valid.astype(f32)[None, :]
        per_graph_mask2[b, :] = (valid[:, None] & valid[None, :]).astype(f32).ravel()
        gm = np.zeros((N, 128), f32)
        for p in range(128):
            gm[int(pairs[b, p, 0]), p] += 0.5
            gm[int(pairs[b, p, 1]), p] += 0.5
        per_graph_G[b] = gm

    shared = {
        "wen_t": np.ascontiguousarray(
            (np.asarray(inputs["Wm_edge"], np.float64)
             @ np.asarray(inputs["W_edge_rs"], np.float64)).T.astype(f32)),
        "w1e_t": np.ascontiguousarray(
            (np.asarray(inputs["link_W1"], np.float64)
             @ np.asarray(inputs["W_edge_rs"], np.float64)).T.astype(f32)),
        "wnr_t": T(inputs["W_node_rs"]),
        "w1_t": T(inputs["link_W1"]), "w2_t": T(inputs["link_W2"]),
        "w3_t": T(inputs["link_W3"]),
        "wmn_t": T(inputs["Wm_node"]),
        "wih_t": T(inputs["Wih"]), "whh_t": T(inputs["Whh"]),
        "lr1_t": T(inputs["lr_W1"]), "lr2_t": T(inputs["lr_W2"]),
        "cr1_t": T(inputs["cr_W1"]), "cr2_t": T(inputs["cr_W2"]),
        "mr1_t": T(inputs["mr_W1"]), "mr2_t": T(inputs["mr_W2"]),
        "ones": np.ones((1, 128), f32), "ident": np.eye(128, dtype=f32),
        "bias_all": _pack_biases(inputs),
    }

    concat = {}
    concat["rsf"] = rsf.reshape(B, NE, EF)          # [16*2? -> (8*2, NE, EF)]
    concat["cnf"] = cnf
    concat["hnmask"] = per_graph_hnmask
    concat["maskv"] = per_graph_maskv
    concat["mask2"] = per_graph_mask2
    concat["gmat"] = per_graph_G
    for k, v in shared.items():
        concat[k] = np.concatenate([v] * NCORES, axis=0)
    return concat


def _postprocess(out_map):
    lr = out_map["out_lr"].reshape(B, 128, 4)
    cr = out_map["out_cr"].reshape(B, 128, 6)
    mr = out_map["out_mr"].reshape(B, 128, 17)
    pa = out_map["out_pa"].reshape(B, N, N)
    return lr, cr, mr, pa


def _run_concat(concat):
    import jax
    r = _get_runner()
    args = [np.ascontiguousarray(concat[n]) for n in r["in_names"]]
    zeros = [np.zeros((NCORES * z.shape[0], *z.shape[1:]), z.dtype)
             for z in r["zero_outs"]]
    outs = r["sharded"](*args, *zeros)
    return {n: np.asarray(outs[i]) for i, n in enumerate(r["out_names"])}


def kernel(**inputs):
    concat = _preprocess(inputs)
    out_map = _run_concat(concat)
    return _postprocess(out_map)


if __name__ == "__main__":
    rng = np.random.default_rng(0)
    print("building...")
    _get_runner()
    print("built ok")


# revision 61
# speedup vs baseline: 1.0549x; 1.0147x over previous
"""GPNN message-passing kernel for 8x Trainium2 NeuronCores.

Strategy:
  - Pure data parallel over batch: B=16 graphs -> 2 graphs per core.
  - Feature-major layout on chip: activations stored [feat_partitions, edges].
  - float32r matmuls (full PE rate at N>=512, ~2e-4 rel err).
  - Edge-state parity trick: E_{t+1}[w,v] = msg_t[v,w] is stored in place
    (no physical transpose); even/odd iterations flip the interpretation of
    the two N axes.  Invalid-pair entries of E are dead values (masked out of
    every output path), so they may hold garbage.
  - w-validity masking folded into hn via -1e30 additive mask before relu.
  - gate broadcast across partitions via a K=1 ones-matmul into PSUM,
    fused relu+gate-multiply via scalar_tensor_tensor.
"""

import os
import sys

for _p in ("/opt/trn_rl_repo",):
    if _p not in sys.path:
        sys.path.insert(0, _p)

import numpy as np

B, N, NF, EF, M, HID = 16, 64, 1024, 256, 256, 512
L_PROP = 3
NCORES = 8
BG = B // NCORES          # graphs per core = 2
NE = N * N                # 4096 edges per graph
ECH = 512                 # edges per chunk
NCH = NE // ECH           # 8 chunks
VCH = ECH // N            # 8 outer-axis rows per chunk
NEG = -1.0e30
A1_WINDOW = 2             # chunks of a1 emitted ahead
NBIAS = 2 + 4 + 2 + 4 + 4 + 1 + 2 + 4 + 2 + 2 + 12 + 3   # packed bias cols = 42

_CACHE = {}


def _build_program():
    import concourse.bass as bass
    import concourse.tile as tile
    from concourse import bacc, mybir

    F32R = mybir.dt.float32r
    FP32 = mybir.dt.float32
    AT = mybir.ActivationFunctionType
    OP = mybir.AluOpType
    AX = mybir.AxisListType

    nc = bacc.Bacc("TRN2", target_bir_lowering=False, debug=False,
                   num_devices=NCORES)

    def din(name, shape, dt=FP32):
        return nc.dram_tensor(name, list(shape), dt, kind="ExternalInput").ap()

    def dout(name, shape, dt=FP32):
        return nc.dram_tensor(name, list(shape), dt, kind="ExternalOutput").ap()

    # ---- inputs ----
    rsf = din("rsf", [BG, NE, EF])               # relative_spatial_feature
    cnf = din("cnf", [BG, N, NF])                # concatenated_node_features
    # weights (pre-transposed host-side to [K, M] layout), f32r
    wen_t = din("wen_t", [EF, M], F32R)
    w1e_t = din("w1e_t", [EF, HID], F32R)
    wnr_t = din("wnr_t", [NF, M], F32R)
    w1_t = din("w1_t", [M, HID], F32R)
    w2_t = din("w2_t", [HID, HID], F32R)
    w3_t = din("w3_t", [HID, 1], F32R)
    wmn_t = din("wmn_t", [M, M], F32R)
    wih_t = din("wih_t", [M, 3 * M], F32R)
    whh_t = din("whh_t", [M, 3 * M], F32R)
    cls_w1 = {k: din(f"{k}1_t", [M, HID], F32R) for k in ("lr", "cr", "mr")}
    cls_od = {"lr": 4, "cr": 6, "mr": 17}
    cls_w2 = {k: din(f"{k}2_t", [HID, cls_od[k]], F32R) for k in cls_od}
    gmat = din("gmat", [BG, N, 128], F32R)       # pair-gather matrix
    ones = din("ones", [1, 128], F32R)
    ident = din("ident", [128, 128])
    # all biases packed column-wise: [128, NBIAS]
    bias_all = din("bias_all", [128, NBIAS])
    # masks
    hnmask = din("hnmask", [BG, 128, N])         # 0 / -1e30 along w
    maskv = din("maskv", [BG, 128, N])           # 1.0 / 0.0 along v
    mask2 = din("mask2", [BG, NE])               # valid pair mask flat

    # ---- outputs ----
    out_lr = dout("out_lr", [BG, 128, 4])
    out_cr = dout("out_cr", [BG, 128, 6])
    out_mr = dout("out_mr", [BG, 128, 17])
    out_pa = dout("out_pa", [BG, NE])

    KM, KH, KNF = M // 128, HID // 128, NF // 128  # 2, 4, 8

    with tile.TileContext(nc, trace_sim=bool(os.environ.get("KTRACE"))) as tc:
        from contextlib import ExitStack
        ctx = ExitStack()
        with ctx:
            wp = ctx.enter_context(tc.tile_pool(name="wp", bufs=1))
            sb = ctx.enter_context(tc.tile_pool(name="sb", bufs=1))
            ps_mm = ctx.enter_context(tc.tile_pool(name="ps_mm", bufs=4, space="PSUM"))
            ps_pd = ctx.enter_context(tc.tile_pool(name="ps_pd", bufs=1, space="PSUM"))
            ps_sm = ctx.enter_context(tc.tile_pool(name="ps_sm", bufs=2, space="PSUM"))

            _wq = [nc.sync, nc.scalar]
            _wqi = [0]

            def wtile(src, k, width, dt=F32R, pfx="w"):
                t = wp.tile([128, width], dt, name=f"{pfx}_{src.name}_{k}")
                _wq[_wqi[0] % 2].dma_start(t[:], src[k * 128:(k + 1) * 128, :])
                _wqi[0] += 1
                return t

            bias_sb = wp.tile([128, NBIAS], FP32, name="bias_sb")
            nc.sync.dma_start(bias_sb[:], bias_all)
            nf_pre = {}
            nf_pre[0] = sb.tile([N, NF], FP32, tag="nf_nm", bufs=1, name="nfnm_pre0")
            nc.sync.dma_start(nf_pre[0][:], cnf[0])
            _bcol = iter(range(NBIAS))
            def bslice(rows=128):
                i = next(_bcol)
                return bias_sb[0:rows, i:i + 1]
            ben_sb = [bslice() for k in range(KM)]
            b1t0_sb = [bslice() for k in range(KH)]
            bnode_sb = [bslice() for k in range(KM)]
            b1_sb = [bslice() for k in range(KH)]
            b2_sb = [bslice() for k in range(KH)]
            b3_sb = bslice(1)
            bm_sb = [bslice() for k in range(KM)]
            brz_sb = [bslice() for k in range(2 * M // 128)]
            bihn_sb = [bslice() for k in range(KM)]
            bhhn_sb = [bslice() for k in range(KM)]
            cb1_sb = {k: [bslice() for j in range(KH)] for k in cls_od}
            cb2_sb = {k: bslice(cls_od[k]) for k in cls_od}
            # ---- load weights (ordered by first use, spread over 2 rings) ----
            id_sb = wp.tile([128, 128], FP32, name="id_sb")
            nc.scalar.dma_start(id_sb[:], ident)
            wen_sb = [wtile(wen_t, k, M) for k in range(KM)]
            w1e_sb = [wtile(w1e_t, k, HID) for k in range(KM)]
            wnr_sb = [wtile(wnr_t, k, M) for k in range(KNF)]
            w1_sb = [wtile(w1_t, k, HID) for k in range(KM)]
            w2_sb = [wtile(w2_t, k, HID) for k in range(KH)]
            w3_sb = [wtile(w3_t, k, 1) for k in range(KH)]
            wmn_sb = [wtile(wmn_t, k, M) for k in range(KM)]
            ones_sb = wp.tile([1, 128], F32R, name="ones_sb")
            nc.scalar.dma_start(ones_sb[:], ones)



            MMOP = dict(start=True, stop=True)
            outmap = {"lr": out_lr, "cr": out_cr, "mr": out_mr}
            H_final = [None] * BG
            ci_sb = [sb.tile([128, BG * 128], F32R, name=f"ci_{k}") for k in range(KM)]

            def make_graph_state(g):
                """Emit phase 0 (node features, H0, masks) and return per-graph
                state with phase-1 emitters."""
                st = {}
                if g in nf_pre:
                    nf_nm = nf_pre[g]
                else:
                    nf_nm = sb.tile([N, NF], FP32, tag="nf_nm", bufs=1, name=f"nfnm{g}")
                    nc.sync.dma_start(nf_nm[:], cnf[g])
                nf_fm = []
                for k8 in range(KNF):
                    pt = ps_mm.tile([128, N], FP32, tag="mm", name=f"nft{g}_{k8}")
                    nc.tensor.transpose(pt[:], nf_nm[:, k8 * 128:(k8 + 1) * 128],
                                        id_sb[0:N, 0:N])
                    t_ = sb.tile([128, N], F32R, tag="nf_fm", bufs=8,
                                 name=f"nffm{g}_{k8}")
                    nc.vector.tensor_copy(t_[:], pt[:])
                    nf_fm.append(t_)
                H_cur = []
                for j in range(KM):
                    ph = ps_mm.tile([128, N], FP32, tag="mm", name=f"h0p{g}_{j}")
                    for k in range(KNF):
                        nc.tensor.matmul(ph[:], wnr_sb[k][:, j * 128:(j + 1) * 128],
                                         nf_fm[k][:], start=(k == 0), stop=(k == KNF - 1))
                    ht = sb.tile([128, N], F32R, tag=f"H{g}_{j}", bufs=2,
                                 name=f"H0_{g}_{j}")
                    nc.scalar.activation(ht[:], ph[:], AT.Identity, bias=bnode_sb[j][:])
                    H_cur.append(ht)
                st["H"] = H_cur
                hnm_sb = sb.tile([128, N], FP32, tag="hnm", bufs=2, name=f"hnm{g}")
                nc.sync.dma_start(hnm_sb[:], hnmask[g])
                mv_sb = sb.tile([128, N], FP32, tag="mv", bufs=2, name=f"mv{g}")
                nc.sync.dma_start(mv_sb[:], maskv[g])
                st["hnm"], st["mv"] = hnm_sb, mv_sb

                E_t = [[None] * NCH for _ in range(KM)]
                en_big = [sb.tile([128, NE], FP32, tag=f"en{j}", name=f"en{g}_{j}")
                          for j in range(KM)]
                st["E"], st["en"] = E_t, en_big
                a1_pre = [None] * NCH
                st["a1_pre"] = a1_pre

                def emit_a1g(c, sfx, wsrc=None, bsrc=None):
                    wsrc = w1_sb if wsrc is None else wsrc
                    bsrc = b1_sb if bsrc is None else bsrc
                    tiles = []
                    for j in range(KH):
                        pa = ps_mm.tile([128, ECH], FP32, tag="mm",
                                        name=f"a1p{g}_{sfx}_{c}_{j}")
                        for k in range(KM):
                            nc.tensor.matmul(pa[:], wsrc[k][:, j * 128:(j + 1) * 128],
                                             E_t[k][c][:], start=(k == 0),
                                             stop=(k == KM - 1))
                        at = sb.tile([128, ECH], F32R, tag="a1", bufs=4 * A1_WINDOW + 2,
                                     name=f"a1{g}_{sfx}_{c}_{j}")
                        nc.scalar.activation(at[:], pa[:], AT.Relu, bias=bsrc[j][:])
                        tiles.append(at)
                    return tiles
                st["emit_a1g"] = emit_a1g

                def ph1a(c, interleaved=False):
                    tp_pool, tp_tag = (ps_sm, "sm") if interleaved else (ps_mm, "mm")
                    for j in range(KM):
                        E_t[j][c] = sb.tile([128, ECH], F32R, tag=f"E{j}_{c}",
                                            name=f"E{g}_{j}_{c}")
                    rnm = sb.tile([128, 4 * EF], FP32, tag="rnm", bufs=3,
                                  name=f"rnm{g}_{c}")
                    nc.sync.dma_start(
                        rnm[:].rearrange("p (et f) -> p et f", et=4),
                        rsf[g, c * ECH:(c + 1) * ECH, :].rearrange(
                            "(et p) f -> p et f", et=4))
                    for et in range(ECH // 128):
                        for j in range(KM):
                            pt = tp_pool.tile([128, 128], FP32, tag=tp_tag,
                                              name=f"rt{g}_{c}_{et}_{j}")
                            o0 = et * EF + j * 128
                            nc.tensor.transpose(pt[:], rnm[:, o0:o0 + 128],
                                                id_sb[:, :])
                            if (et + j) % 2 == 0:
                                nc.vector.tensor_copy(
                                    E_t[j][c][:, et * 128:(et + 1) * 128], pt[:])
                            else:
                                nc.scalar.copy(
                                    E_t[j][c][:, et * 128:(et + 1) * 128], pt[:])
                    if c < A1_WINDOW:
                        a1_pre[c] = emit_a1g(c, "p", wsrc=w1e_sb, bsrc=b1t0_sb)
                st["ph1a"] = ph1a

                def ph1b(c):
                    for j in range(KM):
                        pen = ps_mm.tile([128, ECH], FP32, tag="mm",
                                         name=f"enp{g}_{c}_{j}")
                        for k in range(KM):
                            nc.tensor.matmul(pen[:], wen_sb[k][:, j * 128:(j + 1) * 128],
                                             E_t[k][c][:], start=(k == 0),
                                             stop=(k == KM - 1))
                        nc.vector.tensor_scalar(en_big[j][:, c * ECH:(c + 1) * ECH],
                                                pen[:], ben_sb[j][:], None, op0=OP.add)
                st["ph1b"] = ph1b
                return st

            states = [None] * BG
            states[0] = make_graph_state(0)
            for c in range(NCH):
                states[0]["ph1a"](c)
                states[0]["ph1b"](c)

            # late-needed weights: load during phase 1 compute
            wih_sb = [wtile(wih_t, k, 3 * M) for k in range(KM)]
            whh_sb = [wtile(whh_t, k, 3 * M) for k in range(KM)]
            c1_sb = {k: [wtile(cls_w1[k], j, HID) for j in range(KM)] for k in cls_od}
            c2_sb = {k: [wtile(cls_w2[k], j, cls_od[k]) for j in range(KH)]
                     for k in cls_od}

            for g in range(BG):
                st = states[g]
                E_t, en_big = st["E"], st["en"]
                H_cur, hnm_sb, mv_sb = st["H"], st["hnm"], st["mv"]
                a1_pre, emit_a1g = st["a1_pre"], st["emit_a1g"]
                last = (g == BG - 1)

                for t in range(L_PROP):
                    even = (t % 2 == 0)
                    final_t = (t == L_PROP - 1)
                    hn_m = []
                    for j in range(KM):
                        phn = ps_sm.tile([128, N], FP32, tag="sm", name=f"hnp{g}_{t}_{j}")
                        for k in range(KM):
                            nc.tensor.matmul(phn[:], wmn_sb[k][:, j * 128:(j + 1) * 128],
                                             H_cur[k][:], start=(k == 0), stop=(k == KM - 1))
                        hnt = sb.tile([128, N], FP32, tag="hn", bufs=4,
                                      name=f"hn{g}_{t}_{j}")
                        nc.scalar.activation(hnt[:], phn[:], AT.Identity, bias=bm_sb[j][:])
                        hm = sb.tile([128, N], FP32, tag="hnm2", bufs=4,
                                     name=f"hnm{g}_{t}_{j}")
                        nc.vector.tensor_tensor(hm[:], hnt[:], hnm_sb[:], op=OP.add)
                        hn_m.append(hm)

                    a1_t = [None] * NCH
                    m_fin = [sb.tile([128, N], F32R, tag=f"ms{j}", bufs=2,
                                     name=f"msum{g}_{t}_{j}") for j in range(KM)]
                    mps = None
                    if not even:
                        mps = [sb.tile([128, NCH * N], FP32, tag=f"mps{j}", bufs=1,
                                       name=f"mps{g}_{t}_{j}") for j in range(KM)]
                    if t == 0:
                        for c0 in range(A1_WINDOW):
                            a1_t[c0] = a1_pre[c0]
                    else:
                        for c0 in range(A1_WINDOW):
                            a1_t[c0] = prefetched_a1[c0]

                    prefetched_next = [None] * NCH
                    for c in range(NCH):
                        if c + A1_WINDOW < NCH:
                            if t == 0:
                                a1_t[c + A1_WINDOW] = emit_a1g(
                                    c + A1_WINDOW, "t0", wsrc=w1e_sb, bsrc=b1t0_sb)
                            else:
                                a1_t[c + A1_WINDOW] = emit_a1g(c + A1_WINDOW, f"t{t}")
                        # a2
                        a2_t = []
                        for j in range(KH):
                            pa = ps_mm.tile([128, ECH], FP32, tag="mm",
                                            name=f"a2p{g}_{t}_{c}_{j}")
                            for k in range(KH):
                                nc.tensor.matmul(pa[:], w2_sb[k][:, j * 128:(j + 1) * 128],
                                                 a1_t[c][k][:], start=(k == 0), stop=(k == KH - 1))
                            at = sb.tile([128, ECH], F32R, tag="a2", bufs=4,
                                         name=f"a2{g}_{t}_{c}_{j}")
                            if j < 2:
                                nc.scalar.activation(at[:], pa[:], AT.Relu, bias=b2_sb[j][:])
                            else:
                                nc.vector.tensor_scalar(at[:], pa[:], b2_sb[j][:], 0.0,
                                                        op0=OP.add, op1=OP.max)
                            a2_t.append(at)
                        a1_t[c] = None
                        # padj
                        ppd = ps_pd.tile([1, ECH], FP32, tag="pd", name=f"pdp{g}_{t}_{c}")
                        for k in range(KH):
                            nc.tensor.matmul(ppd[:], w3_sb[k][:], a2_t[k][:],
                                             start=(k == 0), stop=(k == KH - 1))
                        gate = sb.tile([1, ECH], F32R, tag="gate", bufs=2,
                                       name=f"gate{g}_{t}_{c}")
                        nc.scalar.activation(gate[:], ppd[:], AT.Sigmoid, bias=b3_sb[:])
                        if final_t:
                            m2t = sb.tile([1, ECH], FP32, tag="m2", bufs=1,
                                          name=f"m2{g}_{c}")
                            nc.sync.dma_start(
                                m2t[:], mask2[g, c * ECH:(c + 1) * ECH].unsqueeze(0))
                            pam = sb.tile([1, ECH], FP32, tag="pam", bufs=1,
                                          name=f"pam{g}_{c}")
                            nc.vector.scalar_tensor_tensor(
                                pam[:], ppd[:], b3_sb[:], m2t[:],
                                op0=OP.add, op1=OP.mult)
                            nc.sync.dma_start(
                                out_pa[g, c * ECH:(c + 1) * ECH].unsqueeze(0), pam[:])
                        pgb = ps_sm.tile([128, ECH], FP32, tag="gb", bufs=1,
                                         name=f"gb{g}_{t}_{c}")
                        nc.tensor.matmul(pgb[:], ones_sb[:], gate[:], **MMOP)
                        for j in range(KM):
                            tmp = sb.tile([128, ECH], FP32, tag="tmp", bufs=3,
                                          name=f"tmp{g}_{t}_{c}_{j}")
                            tmp3 = tmp[:].rearrange("p (a b) -> p a b", a=VCH)
                            if even:
                                en_v = en_big[j][:, c * ECH:(c + 1) * ECH].rearrange(
                                    "p (a b) -> p a b", a=VCH)
                                hn_v = hn_m[j][:].unsqueeze(1).broadcast_to((128, VCH, N))
                            else:
                                en_v = en_big[j][:].rearrange(
                                    "p (v w) -> p v w", v=N).transpose([0, 2, 1])[
                                    :, c * VCH:(c + 1) * VCH, :]
                                hn_v = hn_m[j][:, c * VCH:(c + 1) * VCH].unsqueeze(
                                    2).broadcast_to((128, VCH, N))
                            nc.vector.tensor_tensor(tmp3, en_v, hn_v, op=OP.add)
                            mdst = tmp if final_t else E_t[j][c]
                            nc.vector.scalar_tensor_tensor(
                                mdst[:], tmp[:], 0.0, pgb[:],
                                op0=OP.max, op1=OP.mult)
                            with nc.allow_low_precision(reason="f32r msum"):
                                if even:
                                    nc.vector.tensor_reduce(
                                        m_fin[j][:, c * VCH:(c + 1) * VCH],
                                        mdst[:].rearrange("p (a b) -> p a b", a=VCH),
                                        axis=AX.X, op=OP.add)
                                else:
                                    nc.vector.tensor_reduce(
                                        mps[j][:, c * N:(c + 1) * N],
                                        mdst[:].rearrange("p (a b) -> p b a", a=VCH),
                                        axis=AX.X, op=OP.add)

                    if t + 1 < L_PROP:
                        for c0 in range(A1_WINDOW):
                            prefetched_next[c0] = emit_a1g(c0, f"t{t + 1}n")
                        prefetched_a1 = prefetched_next
                    if final_t and not last:
                        states[g + 1] = make_graph_state(g + 1)
                        for c_ in range(4):
                            states[g + 1]["ph1a"](c_)
                    if not even:
                        with nc.allow_low_precision(reason="f32r msum"):
                            for j in range(KM):
                                nc.vector.tensor_reduce(
                                    m_fin[j][:],
                                    mps[j][:].rearrange("p (c b) -> p b c", c=NCH),
                                    axis=AX.X, op=OP.add)
                    # ---- GRU ----
                    rz = []
                    for j in range(4):
                        prz = ps_sm.tile([128, N], FP32, tag="sm", name=f"rz{g}_{t}_{j}")
                        for k in range(KM):
                            nc.tensor.matmul(prz[:], wih_sb[k][:, j * 128:(j + 1) * 128],
                                             m_fin[k][:], start=(k == 0), stop=False)
                        for k in range(KM):
                            nc.tensor.matmul(prz[:], whh_sb[k][:, j * 128:(j + 1) * 128],
                                             H_cur[k][:], start=False, stop=(k == KM - 1))
                        rzt = sb.tile([128, N], FP32, tag="rz", bufs=6, name=f"rzt{g}_{t}_{j}")
                        nc.scalar.activation(rzt[:], prz[:], AT.Sigmoid, bias=brz_sb[j][:])
                        rz.append(rzt)
                    H_new = []
                    for j in range(KM):
                        jj = 4 + j
                        pgi = ps_sm.tile([128, N], FP32, tag="sm", name=f"gin{g}_{t}_{j}")
                        for k in range(KM):
                            nc.tensor.matmul(pgi[:], wih_sb[k][:, jj * 128:(jj + 1) * 128],
                                             m_fin[k][:], start=(k == 0), stop=(k == KM - 1))
                        pgh = ps_sm.tile([128, N], FP32, tag="sm", name=f"ghn{g}_{t}_{j}")
                        for k in range(KM):
                            nc.tensor.matmul(pgh[:], whh_sb[k][:, jj * 128:(jj + 1) * 128],
                                             H_cur[k][:], start=(k == 0), stop=(k == KM - 1))
                        s1 = sb.tile([128, N], FP32, tag="s1", bufs=2, name=f"s1{g}_{t}_{j}")
                        nc.scalar.activation(s1[:], pgh[:], AT.Identity, bias=bhhn_sb[j][:])
                        s2 = sb.tile([128, N], FP32, tag="s2", bufs=2, name=f"s2{g}_{t}_{j}")
                        nc.vector.tensor_tensor(s2[:], rz[j][:], s1[:], op=OP.mult)
                        s3 = sb.tile([128, N], FP32, tag="s3", bufs=2, name=f"s3{g}_{t}_{j}")
                        nc.vector.tensor_tensor(s3[:], s2[:], pgi[:], op=OP.add)
                        nn = sb.tile([128, N], FP32, tag="nn", bufs=2, name=f"nn{g}_{t}_{j}")
                        nc.scalar.activation(nn[:], s3[:], AT.Tanh, bias=bihn_sb[j][:])
                        d1 = sb.tile([128, N], FP32, tag="d1", bufs=2, name=f"d1{g}_{t}_{j}")
                        nc.vector.tensor_tensor(d1[:], H_cur[j][:], nn[:], op=OP.subtract)
                        zd = sb.tile([128, N], FP32, tag="zd", bufs=2, name=f"zd{g}_{t}_{j}")
                        nc.vector.tensor_tensor(zd[:], rz[2 + j][:], d1[:], op=OP.mult)
                        hp = sb.tile([128, N], FP32, tag="hp", bufs=2, name=f"hp{g}_{t}_{j}")
                        nc.vector.tensor_tensor(hp[:], nn[:], zd[:], op=OP.add)
                        d2 = sb.tile([128, N], FP32, tag="d2", bufs=2, name=f"d2{g}_{t}_{j}")
                        nc.vector.tensor_tensor(d2[:], hp[:], H_cur[j][:], op=OP.subtract)
                        md = sb.tile([128, N], FP32, tag="md", bufs=2, name=f"md{g}_{t}_{j}")
                        nc.vector.tensor_tensor(md[:], mv_sb[:], d2[:], op=OP.mult)
                        hnw = sb.tile([128, N], F32R, tag=f"H{g}_{j}", bufs=2,
                                      name=f"H{g}_{t + 1}_{j}")
                        nc.vector.tensor_tensor(hnw[:], H_cur[j][:], md[:], op=OP.add)
                        H_new.append(hnw)
                    H_cur = H_new

                H_final[g] = H_cur
                if not last:
                    for c_ in range(4, NCH):
                        states[g + 1]["ph1a"](c_)
                    for c_ in range(NCH):
                        states[g + 1]["ph1b"](c_)

            # ---- pair gather (both graphs) ----
            for g in range(BG):
                H_cur = H_final[g]
                h_nm = sb.tile([N, M], F32R, tag="h_nm", bufs=2, name=f"hnm_t{g}")
                for j in range(KM):
                    pt = ps_sm.tile([N, 128], FP32, tag="sm", name=f"htr{g}_{j}")
                    nc.tensor.transpose(pt[:], H_cur[j][:].bitcast(FP32), id_sb[:, :])
                    nc.vector.tensor_copy(h_nm[:, j * 128:(j + 1) * 128], pt[:])
                gt = sb.tile([N, 128], F32R, tag="gt", bufs=2, name=f"gt{g}")
                nc.sync.dma_start(gt[:], gmat[g])
                for j in range(KM):
                    pci = ps_mm.tile([128, 128], FP32, tag="mm", name=f"ci{g}_{j}")
                    nc.tensor.matmul(pci[:], h_nm[:, j * 128:(j + 1) * 128], gt[:], **MMOP)
                    nc.vector.tensor_copy(ci_sb[j][:, g * 128:(g + 1) * 128], pci[:])

            # ================= phase 4: classifiers (batched) =================
            for key in ("lr", "cr", "mr"):
                od = cls_od[key]
                h1 = []
                for j in range(KH):
                    ph = ps_mm.tile([128, BG * 128], FP32, tag="mm", name=f"c1{key}_{j}")
                    for k in range(KM):
                        nc.tensor.matmul(ph[:], c1_sb[key][k][:, j * 128:(j + 1) * 128],
                                         ci_sb[k][:], start=(k == 0), stop=(k == KM - 1))
                    ht = sb.tile([128, BG * 128], F32R, tag=f"h1{key}", bufs=2,
                                 name=f"h1{key}_{j}")
                    nc.scalar.activation(ht[:], ph[:], AT.Relu, bias=cb1_sb[key][j][:])
                    h1.append(ht)
                po = ps_mm.tile([od, BG * 128], FP32, tag="mm", name=f"c2{key}")
                for k in range(KH):
                    nc.tensor.matmul(po[:], c2_sb[key][k][:], h1[k][:],
                                     start=(k == 0), stop=(k == KH - 1))
                osb = sb.tile([od, BG * 128], FP32, tag="osb", bufs=2, name=f"osb{key}")
                nc.scalar.activation(osb[:], po[:], AT.Identity, bias=cb2_sb[key][:])
                for g in range(BG):
                    ptr = ps_sm.tile([128, od], FP32, tag="sm", name=f"otr{key}_{g}")
                    nc.tensor.transpose(ptr[:], osb[:, g * 128:(g + 1) * 128],
                                        id_sb[0:od, 0:od])
                    og = sb.tile([128, od], FP32, tag="og", bufs=2, name=f"og{key}_{g}")
                    nc.vector.tensor_copy(og[:], ptr[:])
                    nc.sync.dma_start(outmap[key][g], og[:])

    nc.compile()
    return nc


def _get_runner():
    if "runner" in _CACHE:
        return _CACHE["runner"]
    import jax
    import numpy as np
    from jax.experimental.shard_map import shard_map
    from jax.sharding import Mesh, NamedSharding, PartitionSpec
    from concourse import mybir
    from concourse.bass2jax import (_bass_exec_p, install_neuronx_cc_hook,
                                    partition_id_tensor)

    nc = _build_program()
    _CACHE["nc"] = nc
    install_neuronx_cc_hook()

    pname = nc.partition_id_tensor.name if nc.partition_id_tensor else None
    in_names, out_names, out_avals, zero_outs = [], [], [], []
    for alloc in nc.m.functions[0].allocations:
        if not isinstance(alloc, mybir.MemoryLocationSet):
            continue
        name = alloc.memorylocations[0].name
        if alloc.kind == "ExternalInput":
            if name != pname:
                in_names.append(name)
        elif alloc.kind == "ExternalOutput":
            out_names.append(name)
            shape = tuple(alloc.tensor_shape)
            dtype = mybir.dt.np(alloc.dtype)
            out_avals.append(jax.core.ShapedArray(shape, dtype))
            zero_outs.append(np.zeros(shape, dtype))
    n_params = len(in_names)
    all_in_names = in_names + out_names
    if pname is not None:
        all_in_names = all_in_names + [pname]

    def _body(*args):
        operands = list(args)
        if pname is not None:
            operands.append(partition_id_tensor())
        outs = _bass_exec_p.bind(
            *operands,
            out_avals=tuple(out_avals),
            in_names=tuple(all_in_names),
            out_names=tuple(out_names),
            lowering_input_output_aliases=(),
            sim_require_finite=False,
            sim_require_nnan=False,
            nc=nc,
        )
        return tuple(outs)

    devices = jax.devices()[:NCORES]
    mesh = Mesh(np.asarray(devices), ("core",))
    n_all = n_params + len(zero_outs)
    sharded = jax.jit(
        shard_map(_body, mesh=mesh,
                  in_specs=(PartitionSpec("core"),) * n_all,
                  out_specs=(PartitionSpec("core"),) * len(out_names),
                  check_rep=False),
        keep_unused=True,
    )
    sharding = NamedSharding(mesh, PartitionSpec("core"))
    runner = dict(sharded=sharded, in_names=in_names, out_names=out_names,
                  out_avals=out_avals, zero_outs=zero_outs, sharding=sharding,
                  mesh=mesh)
    _CACHE["runner"] = runner
    return runner


def _pack_biases(inputs):
    f32 = np.float32
    cols = []

    def add(vec, chunks=None):
        v = np.asarray(vec, f32).ravel()
        n = (len(v) + 127) // 128 if chunks is None else chunks
        for k in range(n):
            c = np.zeros(128, f32)
            seg = v[k * 128:(k + 1) * 128]
            c[:len(seg)] = seg
            cols.append(c)

    be = np.asarray(inputs["b_edge_rs"], np.float64)
    add(np.asarray(inputs["Wm_edge"], np.float64) @ be)          # b_en: 2
    add(np.asarray(inputs["link_W1"], np.float64) @ be
        + np.asarray(inputs["link_b1"], np.float64))             # b1t0: 4
    add(inputs["b_node_rs"])                      # 2
    add(inputs["link_b1"])                        # 4
    add(inputs["link_b2"])                        # 4
    add(inputs["link_b3"])                        # 1
    add(inputs["bm"])                             # 2
    add(np.asarray(inputs["bih"], f32)[0:2 * M]
        + np.asarray(inputs["bhh"], f32)[0:2 * M])  # 4
    add(np.asarray(inputs["bih"], f32)[2 * M:])   # 2
    add(np.asarray(inputs["bhh"], f32)[2 * M:])   # 2
    for k in ("lr", "cr", "mr"):
        add(inputs[f"{k}_b1"])                    # 4 each
    for k in ("lr", "cr", "mr"):
        add(inputs[f"{k}_b2"], chunks=1)          # 1 each
    out = np.stack(cols, axis=1)
    assert out.shape[1] == NBIAS, out.shape
    return np.ascontiguousarray(out)


def _preprocess(inputs):
    """Host-side prep: per-core input dict values, each shaped [ncores*d0, ...]."""
    f32 = np.float32
    rsf = np.ascontiguousarray(inputs["relative_spatial_feature"], f32)
    cnf = np.ascontiguousarray(inputs["concatenated_node_features"], f32)
    num_obj = np.asarray(inputs["num_obj"])
    pairs = np.asarray(inputs["object_pairs"])

    T = lambda a: np.ascontiguousarray(np.asarray(a, f32).T)
    col = lambda a: np.ascontiguousarray(np.asarray(a, f32).reshape(-1, 1))

    per_graph_hnmask = np.zeros((B, 128, N), f32)
    per_graph_maskv = np.zeros((B, 128, N), f32)
    per_graph_mask2 = np.zeros((B, NE), f32)
    per_graph_G = np.zeros((B, N, 128), f32)
    for b in range(B):
        valid = (np.arange(N) < int(num_obj[b]))
        per_graph_hnmask[b, :, :] = np.where(valid, 0.0, NEG)[None, :]
        per_graph_maskv[b, :, :] = valid.astype(f32)[None, :]
        per_graph_mask2[b, :] = (valid[:, None] & valid[None, :]).astype(f32).ravel()
        gm = np.zeros((N, 128), f32)
        for p in range(128):
            gm[int(pairs[b, p, 0]), p] += 0.5
            gm[int(pairs[b, p, 1]), p] += 0.5
        per_graph_G[b] = gm

    shared = {
        "wen_t": np.ascontiguousarray(
            (np.asarray(inputs["Wm_edge"], np.float64)
             @ np.asarray(inputs["W_edge_rs"], np.float64)).T.astype(f32)),
        "w1e_t": np.ascontiguousarray(
            (np.asarray(inputs["link_W1"], np.float64)
             @ np.asarray(inputs["W_edge_rs"], np.float64)).T.astype(f32)),
        "wnr_t": T(inputs["W_node_rs"]),
        "w1_t": T(inputs["link_W1"]), "w2_t": T(inputs["link_W2"]),
        "w3_t": T(inputs["link_W3"]),
        "wmn_t": T(inputs["Wm_node"]),
        "wih_t": T(inputs["Wih"]), "whh_t": T(inputs["Whh"]),
        "lr1_t": T(inputs["lr_W1"]), "lr2_t": T(inputs["lr_W2"]),
        "cr1_t": T(inputs["cr_W1"]), "cr2_t": T(inputs["cr_W2"]),
        "mr1_t": T(inputs["mr_W1"]), "mr2_t": T(inputs["mr_W2"]),
        "ones": np.ones((1, 128), f32), "ident": np.eye(128, dtype=f32),
        "bias_all": _pack_biases(inputs),
    }

    concat = {}
    concat["rsf"] = rsf.reshape(B, NE, EF)          # [16*2? -> (8*2, NE, EF)]
    concat["cnf"] = cnf
    concat["hnmask"] = per_graph_hnmask
    concat["maskv"] = per_graph_maskv
    concat["mask2"] = per_graph_mask2
    concat["gmat"] = per_graph_G
    for k, v in shared.items():
        concat[k] = np.concatenate([v] * NCORES, axis=0)
    return concat


def _postprocess(out_map):
    lr = out_map["out_lr"].reshape(B, 128, 4)
    cr = out_map["out_cr"].reshape(B, 128, 6)
    mr = out_map["out_mr"].reshape(B, 128, 17)
    pa = out_map["out_pa"].reshape(B, N, N)
    return lr, cr, mr, pa


def _run_concat(concat):
    import jax
    r = _get_runner()
    args = [np.ascontiguousarray(concat[n]) for n in r["in_names"]]
    zeros = [np.zeros((NCORES * z.shape[0], *z.shape[1:]), z.dtype)
             for z in r["zero_outs"]]
    outs = r["sharded"](*args, *zeros)
    return {n: np.asarray(outs[i]) for i, n in enumerate(r["out_names"])}


def kernel(**inputs):
    concat = _preprocess(inputs)
    out_map = _run_concat(concat)
    return _postprocess(out_map)


if __name__ == "__main__":
    rng = np.random.default_rng(0)
    print("building...")
    _get_runner()
    print("built ok")


# revision 62
# speedup vs baseline: 1.0618x; 1.0066x over previous
"""GPNN message-passing kernel for 8x Trainium2 NeuronCores.

Strategy:
  - Pure data parallel over batch: B=16 graphs -> 2 graphs per core.
  - Feature-major layout on chip: activations stored [feat_partitions, edges].
  - float32r matmuls (full PE rate at N>=512, ~2e-4 rel err).
  - Edge-state parity trick: E_{t+1}[w,v] = msg_t[v,w] is stored in place
    (no physical transpose); even/odd iterations flip the interpretation of
    the two N axes.  Invalid-pair entries of E are dead values (masked out of
    every output path), so they may hold garbage.
  - w-validity masking folded into hn via -1e30 additive mask before relu.
  - gate broadcast across partitions via a K=1 ones-matmul into PSUM,
    fused relu+gate-multiply via scalar_tensor_tensor.
"""

import os
import sys

for _p in ("/opt/trn_rl_repo",):
    if _p not in sys.path:
        sys.path.insert(0, _p)

import numpy as np

B, N, NF, EF, M, HID = 16, 64, 1024, 256, 256, 512
L_PROP = 3
NCORES = 8
BG = B // NCORES          # graphs per core = 2
NE = N * N                # 4096 edges per graph
ECH = 512                 # edges per chunk
NCH = NE // ECH           # 8 chunks
VCH = ECH // N            # 8 outer-axis rows per chunk
NEG = -1.0e30
A1_WINDOW = 2             # chunks of a1 emitted ahead
NBIAS = 2 + 4 + 2 + 4 + 4 + 1 + 2 + 4 + 2 + 2 + 12 + 3   # packed bias cols = 42

_CACHE = {}


def _build_program():
    import concourse.bass as bass
    import concourse.tile as tile
    from concourse import bacc, mybir

    F32R = mybir.dt.float32r
    FP32 = mybir.dt.float32
    AT = mybir.ActivationFunctionType
    OP = mybir.AluOpType
    AX = mybir.AxisListType

    nc = bacc.Bacc("TRN2", target_bir_lowering=False, debug=False,
                   num_devices=NCORES)

    def din(name, shape, dt=FP32):
        return nc.dram_tensor(name, list(shape), dt, kind="ExternalInput").ap()

    def dout(name, shape, dt=FP32):
        return nc.dram_tensor(name, list(shape), dt, kind="ExternalOutput").ap()

    # ---- inputs ----
    rsf = din("rsf", [BG, NE, EF])               # relative_spatial_feature
    cnf = din("cnf", [BG, N, NF])                # concatenated_node_features
    # weights (pre-transposed host-side to [K, M] layout), f32r
    wen_t = din("wen_t", [EF, M], F32R)
    w1e_t = din("w1e_t", [EF, HID], F32R)
    wnr_t = din("wnr_t", [NF, M], F32R)
    w1_t = din("w1_t", [M, HID], F32R)
    w2_t = din("w2_t", [HID, HID], F32R)
    w3_t = din("w3_t", [HID, 1], F32R)
    wmn_t = din("wmn_t", [M, M], F32R)
    wih_t = din("wih_t", [M, 3 * M], F32R)
    whh_t = din("whh_t", [M, 3 * M], F32R)
    cls_w1 = {k: din(f"{k}1_t", [M, HID], F32R) for k in ("lr", "cr", "mr")}
    cls_od = {"lr": 4, "cr": 6, "mr": 17}
    cls_w2 = {k: din(f"{k}2_t", [HID, cls_od[k]], F32R) for k in cls_od}
    gmat = din("gmat", [BG, N, 128], F32R)       # pair-gather matrix
    ones = din("ones", [1, 128], F32R)
    ident = din("ident", [128, 128])
    # all biases packed column-wise: [128, NBIAS]
    bias_all = din("bias_all", [128, NBIAS])
    # masks
    hnmask = din("hnmask", [BG, 128, N])         # 0 / -1e30 along w
    maskv = din("maskv", [BG, 128, N])           # 1.0 / 0.0 along v
    mask2 = din("mask2", [BG, NE])               # valid pair mask flat

    # ---- outputs ----
    out_lr = dout("out_lr", [BG, 128, 4])
    out_cr = dout("out_cr", [BG, 128, 6])
    out_mr = dout("out_mr", [BG, 128, 17])
    out_pa = dout("out_pa", [BG, NE])

    KM, KH, KNF = M // 128, HID // 128, NF // 128  # 2, 4, 8

    with tile.TileContext(nc, trace_sim=bool(os.environ.get("KTRACE"))) as tc:
        from contextlib import ExitStack
        ctx = ExitStack()
        with ctx:
            wp = ctx.enter_context(tc.tile_pool(name="wp", bufs=1))
            sb = ctx.enter_context(tc.tile_pool(name="sb", bufs=1))
            ps_mm = ctx.enter_context(tc.tile_pool(name="ps_mm", bufs=4, space="PSUM"))
            ps_pd = ctx.enter_context(tc.tile_pool(name="ps_pd", bufs=1, space="PSUM"))
            ps_sm = ctx.enter_context(tc.tile_pool(name="ps_sm", bufs=2, space="PSUM"))

            _wq = [nc.sync, nc.scalar]
            _wqi = [0]

            def wtile(src, k, width, dt=F32R, pfx="w"):
                t = wp.tile([128, width], dt, name=f"{pfx}_{src.name}_{k}")
                _wq[_wqi[0] % 2].dma_start(t[:], src[k * 128:(k + 1) * 128, :])
                _wqi[0] += 1
                return t

            bias_sb = wp.tile([128, NBIAS], FP32, name="bias_sb")
            nc.sync.dma_start(bias_sb[:], bias_all)
            nf_pre = {}
            nf_pre[0] = sb.tile([N, NF], FP32, tag="nf_nm", bufs=1, name="nfnm_pre0")
            nc.sync.dma_start(nf_pre[0][:], cnf[0])
            _bcol = iter(range(NBIAS))
            def bslice(rows=128):
                i = next(_bcol)
                return bias_sb[0:rows, i:i + 1]
            ben_sb = [bslice() for k in range(KM)]
            b1t0_sb = [bslice() for k in range(KH)]
            bnode_sb = [bslice() for k in range(KM)]
            b1_sb = [bslice() for k in range(KH)]
            b2_sb = [bslice() for k in range(KH)]
            b3_sb = bslice(1)
            bm_sb = [bslice() for k in range(KM)]
            brz_sb = [bslice() for k in range(2 * M // 128)]
            bihn_sb = [bslice() for k in range(KM)]
            bhhn_sb = [bslice() for k in range(KM)]
            cb1_sb = {k: [bslice() for j in range(KH)] for k in cls_od}
            cb2_sb = {k: bslice(cls_od[k]) for k in cls_od}
            # ---- load weights (ordered by first use, spread over 2 rings) ----
            id_sb = wp.tile([128, 128], FP32, name="id_sb")
            nc.scalar.dma_start(id_sb[:], ident)
            wen_sb = [wtile(wen_t, k, M) for k in range(KM)]
            w1e_sb = [wtile(w1e_t, k, HID) for k in range(KM)]
            wnr_sb = [wtile(wnr_t, k, M) for k in range(KNF)]
            w1_sb = [wtile(w1_t, k, HID) for k in range(KM)]
            w2_sb = [wtile(w2_t, k, HID) for k in range(KH)]
            w3_sb = [wtile(w3_t, k, 1) for k in range(KH)]
            wmn_sb = [wtile(wmn_t, k, M) for k in range(KM)]
            ones_sb = wp.tile([1, 128], F32R, name="ones_sb")
            nc.scalar.dma_start(ones_sb[:], ones)



            MMOP = dict(start=True, stop=True)
            outmap = {"lr": out_lr, "cr": out_cr, "mr": out_mr}
            H_final = [None] * BG
            ci_sb = [sb.tile([128, BG * 128], F32R, name=f"ci_{k}") for k in range(KM)]

            def make_graph_state(g):
                """Emit phase 0 (node features, H0, masks) and return per-graph
                state with phase-1 emitters."""
                st = {}
                if g in nf_pre:
                    nf_nm = nf_pre[g]
                else:
                    nf_nm = sb.tile([N, NF], FP32, tag="nf_nm", bufs=1, name=f"nfnm{g}")
                    nc.sync.dma_start(nf_nm[:], cnf[g])
                nf_fm = []
                for k8 in range(KNF):
                    pt = ps_mm.tile([128, N], FP32, tag="mm", name=f"nft{g}_{k8}")
                    nc.tensor.transpose(pt[:], nf_nm[:, k8 * 128:(k8 + 1) * 128],
                                        id_sb[0:N, 0:N])
                    t_ = sb.tile([128, N], F32R, tag="nf_fm", bufs=8,
                                 name=f"nffm{g}_{k8}")
                    nc.vector.tensor_copy(t_[:], pt[:])
                    nf_fm.append(t_)
                H_cur = []
                for j in range(KM):
                    ph = ps_mm.tile([128, N], FP32, tag="mm", name=f"h0p{g}_{j}")
                    for k in range(KNF):
                        nc.tensor.matmul(ph[:], wnr_sb[k][:, j * 128:(j + 1) * 128],
                                         nf_fm[k][:], start=(k == 0), stop=(k == KNF - 1))
                    ht = sb.tile([128, N], F32R, tag=f"H{g}_{j}", bufs=2,
                                 name=f"H0_{g}_{j}")
                    nc.scalar.activation(ht[:], ph[:], AT.Identity, bias=bnode_sb[j][:])
                    H_cur.append(ht)
                st["H"] = H_cur
                hnm_sb = sb.tile([128, N], FP32, tag="hnm", bufs=2, name=f"hnm{g}")
                nc.sync.dma_start(hnm_sb[:], hnmask[g])
                mv_sb = sb.tile([128, N], FP32, tag="mv", bufs=2, name=f"mv{g}")
                nc.sync.dma_start(mv_sb[:], maskv[g])
                st["hnm"], st["mv"] = hnm_sb, mv_sb

                E_t = [[None] * NCH for _ in range(KM)]
                en_big = [sb.tile([128, NE], FP32, tag=f"en{j}", name=f"en{g}_{j}")
                          for j in range(KM)]
                st["E"], st["en"] = E_t, en_big
                a1_pre = [None] * NCH
                st["a1_pre"] = a1_pre

                def emit_a1g(c, sfx, wsrc=None, bsrc=None):
                    wsrc = w1_sb if wsrc is None else wsrc
                    bsrc = b1_sb if bsrc is None else bsrc
                    tiles = []
                    for j in range(KH):
                        pa = ps_mm.tile([128, ECH], FP32, tag="mm",
                                        name=f"a1p{g}_{sfx}_{c}_{j}")
                        for k in range(KM):
                            nc.tensor.matmul(pa[:], wsrc[k][:, j * 128:(j + 1) * 128],
                                             E_t[k][c][:], start=(k == 0),
                                             stop=(k == KM - 1))
                        at = sb.tile([128, ECH], F32R, tag="a1", bufs=4 * A1_WINDOW + 2,
                                     name=f"a1{g}_{sfx}_{c}_{j}")
                        nc.scalar.activation(at[:], pa[:], AT.Relu, bias=bsrc[j][:])
                        tiles.append(at)
                    return tiles
                st["emit_a1g"] = emit_a1g

                def ph1a(c, interleaved=False):
                    tp_pool, tp_tag = (ps_sm, "sm") if interleaved else (ps_mm, "mm")
                    for j in range(KM):
                        E_t[j][c] = sb.tile([128, ECH], F32R, tag=f"E{j}_{c}",
                                            name=f"E{g}_{j}_{c}")
                    rnm = sb.tile([128, 4 * EF], FP32, tag="rnm", bufs=3,
                                  name=f"rnm{g}_{c}")
                    nc.sync.dma_start(
                        rnm[:].rearrange("p (et f) -> p et f", et=4),
                        rsf[g, c * ECH:(c + 1) * ECH, :].rearrange(
                            "(et p) f -> p et f", et=4))
                    for et in range(ECH // 128):
                        for j in range(KM):
                            pt = tp_pool.tile([128, 128], FP32, tag=tp_tag,
                                              name=f"rt{g}_{c}_{et}_{j}")
                            o0 = et * EF + j * 128
                            nc.tensor.transpose(pt[:], rnm[:, o0:o0 + 128],
                                                id_sb[:, :])
                            if (et + j) % 2 == 0:
                                nc.vector.tensor_copy(
                                    E_t[j][c][:, et * 128:(et + 1) * 128], pt[:])
                            else:
                                nc.scalar.copy(
                                    E_t[j][c][:, et * 128:(et + 1) * 128], pt[:])
                    if c < A1_WINDOW:
                        a1_pre[c] = emit_a1g(c, "p", wsrc=w1e_sb, bsrc=b1t0_sb)
                st["ph1a"] = ph1a

                def ph1b(c):
                    for j in range(KM):
                        pen = ps_mm.tile([128, ECH], FP32, tag="mm",
                                         name=f"enp{g}_{c}_{j}")
                        for k in range(KM):
                            nc.tensor.matmul(pen[:], wen_sb[k][:, j * 128:(j + 1) * 128],
                                             E_t[k][c][:], start=(k == 0),
                                             stop=(k == KM - 1))
                        if j == 0:
                            nc.scalar.activation(en_big[j][:, c * ECH:(c + 1) * ECH],
                                                 pen[:], AT.Identity, bias=ben_sb[j][:])
                        else:
                            nc.vector.tensor_scalar(en_big[j][:, c * ECH:(c + 1) * ECH],
                                                    pen[:], ben_sb[j][:], None, op0=OP.add)
                st["ph1b"] = ph1b
                return st

            states = [None] * BG
            states[0] = make_graph_state(0)
            for c in range(NCH):
                states[0]["ph1a"](c)
                states[0]["ph1b"](c)

            # late-needed weights: load during phase 1 compute
            wih_sb = [wtile(wih_t, k, 3 * M) for k in range(KM)]
            whh_sb = [wtile(whh_t, k, 3 * M) for k in range(KM)]
            c1_sb = {k: [wtile(cls_w1[k], j, HID) for j in range(KM)] for k in cls_od}
            c2_sb = {k: [wtile(cls_w2[k], j, cls_od[k]) for j in range(KH)]
                     for k in cls_od}

            for g in range(BG):
                st = states[g]
                E_t, en_big = st["E"], st["en"]
                H_cur, hnm_sb, mv_sb = st["H"], st["hnm"], st["mv"]
                a1_pre, emit_a1g = st["a1_pre"], st["emit_a1g"]
                last = (g == BG - 1)

                for t in range(L_PROP):
                    even = (t % 2 == 0)
                    final_t = (t == L_PROP - 1)
                    hn_m = []
                    for j in range(KM):
                        phn = ps_sm.tile([128, N], FP32, tag="sm", name=f"hnp{g}_{t}_{j}")
                        for k in range(KM):
                            nc.tensor.matmul(phn[:], wmn_sb[k][:, j * 128:(j + 1) * 128],
                                             H_cur[k][:], start=(k == 0), stop=(k == KM - 1))
                        hnt = sb.tile([128, N], FP32, tag="hn", bufs=4,
                                      name=f"hn{g}_{t}_{j}")
                        nc.scalar.activation(hnt[:], phn[:], AT.Identity, bias=bm_sb[j][:])
                        hm = sb.tile([128, N], FP32, tag="hnm2", bufs=4,
                                     name=f"hnm{g}_{t}_{j}")
                        nc.vector.tensor_tensor(hm[:], hnt[:], hnm_sb[:], op=OP.add)
                        hn_m.append(hm)

                    a1_t = [None] * NCH
                    m_fin = [sb.tile([128, N], F32R, tag=f"ms{j}", bufs=2,
                                     name=f"msum{g}_{t}_{j}") for j in range(KM)]
                    mps = None
                    if not even:
                        mps = [sb.tile([128, NCH * N], FP32, tag=f"mps{j}", bufs=1,
                                       name=f"mps{g}_{t}_{j}") for j in range(KM)]
                    if t == 0:
                        for c0 in range(A1_WINDOW):
                            a1_t[c0] = a1_pre[c0]
                    else:
                        for c0 in range(A1_WINDOW):
                            a1_t[c0] = prefetched_a1[c0]

                    prefetched_next = [None] * NCH
                    for c in range(NCH):
                        if c + A1_WINDOW < NCH:
                            if t == 0:
                                a1_t[c + A1_WINDOW] = emit_a1g(
                                    c + A1_WINDOW, "t0", wsrc=w1e_sb, bsrc=b1t0_sb)
                            else:
                                a1_t[c + A1_WINDOW] = emit_a1g(c + A1_WINDOW, f"t{t}")
                        # a2
                        a2_t = []
                        for j in range(KH):
                            pa = ps_mm.tile([128, ECH], FP32, tag="mm",
                                            name=f"a2p{g}_{t}_{c}_{j}")
                            for k in range(KH):
                                nc.tensor.matmul(pa[:], w2_sb[k][:, j * 128:(j + 1) * 128],
                                                 a1_t[c][k][:], start=(k == 0), stop=(k == KH - 1))
                            at = sb.tile([128, ECH], F32R, tag="a2", bufs=4,
                                         name=f"a2{g}_{t}_{c}_{j}")
                            if j < 2:
                                nc.scalar.activation(at[:], pa[:], AT.Relu, bias=b2_sb[j][:])
                            else:
                                nc.vector.tensor_scalar(at[:], pa[:], b2_sb[j][:], 0.0,
                                                        op0=OP.add, op1=OP.max)
                            a2_t.append(at)
                        a1_t[c] = None
                        # padj
                        ppd = ps_pd.tile([1, ECH], FP32, tag="pd", name=f"pdp{g}_{t}_{c}")
                        for k in range(KH):
                            nc.tensor.matmul(ppd[:], w3_sb[k][:], a2_t[k][:],
                                             start=(k == 0), stop=(k == KH - 1))
                        gate = sb.tile([1, ECH], F32R, tag="gate", bufs=2,
                                       name=f"gate{g}_{t}_{c}")
                        nc.scalar.activation(gate[:], ppd[:], AT.Sigmoid, bias=b3_sb[:])
                        if final_t:
                            m2t = sb.tile([1, ECH], FP32, tag="m2", bufs=1,
                                          name=f"m2{g}_{c}")
                            nc.sync.dma_start(
                                m2t[:], mask2[g, c * ECH:(c + 1) * ECH].unsqueeze(0))
                            pam = sb.tile([1, ECH], FP32, tag="pam", bufs=1,
                                          name=f"pam{g}_{c}")
                            nc.vector.scalar_tensor_tensor(
                                pam[:], ppd[:], b3_sb[:], m2t[:],
                                op0=OP.add, op1=OP.mult)
                            nc.sync.dma_start(
                                out_pa[g, c * ECH:(c + 1) * ECH].unsqueeze(0), pam[:])
                        pgb = ps_sm.tile([128, ECH], FP32, tag="gb", bufs=1,
                                         name=f"gb{g}_{t}_{c}")
                        nc.tensor.matmul(pgb[:], ones_sb[:], gate[:], **MMOP)
                        for j in range(KM):
                            tmp = sb.tile([128, ECH], FP32, tag="tmp", bufs=3,
                                          name=f"tmp{g}_{t}_{c}_{j}")
                            tmp3 = tmp[:].rearrange("p (a b) -> p a b", a=VCH)
                            if even:
                                en_v = en_big[j][:, c * ECH:(c + 1) * ECH].rearrange(
                                    "p (a b) -> p a b", a=VCH)
                                hn_v = hn_m[j][:].unsqueeze(1).broadcast_to((128, VCH, N))
                            else:
                                en_v = en_big[j][:].rearrange(
                                    "p (v w) -> p v w", v=N).transpose([0, 2, 1])[
                                    :, c * VCH:(c + 1) * VCH, :]
                                hn_v = hn_m[j][:, c * VCH:(c + 1) * VCH].unsqueeze(
                                    2).broadcast_to((128, VCH, N))
                            nc.vector.tensor_tensor(tmp3, en_v, hn_v, op=OP.add)
                            mdst = tmp if final_t else E_t[j][c]
                            nc.vector.scalar_tensor_tensor(
                                mdst[:], tmp[:], 0.0, pgb[:],
                                op0=OP.max, op1=OP.mult)
                            with nc.allow_low_precision(reason="f32r msum"):
                                if even:
                                    nc.vector.tensor_reduce(
                                        m_fin[j][:, c * VCH:(c + 1) * VCH],
                                        mdst[:].rearrange("p (a b) -> p a b", a=VCH),
                                        axis=AX.X, op=OP.add)
                                else:
                                    nc.vector.tensor_reduce(
                                        mps[j][:, c * N:(c + 1) * N],
                                        mdst[:].rearrange("p (a b) -> p b a", a=VCH),
                                        axis=AX.X, op=OP.add)

                    if t + 1 < L_PROP:
                        for c0 in range(A1_WINDOW):
                            prefetched_next[c0] = emit_a1g(c0, f"t{t + 1}n")
                        prefetched_a1 = prefetched_next
                    if final_t and not last:
                        states[g + 1] = make_graph_state(g + 1)
                        for c_ in range(4):
                            states[g + 1]["ph1a"](c_)
                    if not even:
                        with nc.allow_low_precision(reason="f32r msum"):
                            for j in range(KM):
                                nc.vector.tensor_reduce(
                                    m_fin[j][:],
                                    mps[j][:].rearrange("p (c b) -> p b c", c=NCH),
                                    axis=AX.X, op=OP.add)
                    # ---- GRU ----
                    rz = []
                    for j in range(4):
                        prz = ps_sm.tile([128, N], FP32, tag="sm", name=f"rz{g}_{t}_{j}")
                        for k in range(KM):
                            nc.tensor.matmul(prz[:], wih_sb[k][:, j * 128:(j + 1) * 128],
                                             m_fin[k][:], start=(k == 0), stop=False)
                        for k in range(KM):
                            nc.tensor.matmul(prz[:], whh_sb[k][:, j * 128:(j + 1) * 128],
                                             H_cur[k][:], start=False, stop=(k == KM - 1))
                        rzt = sb.tile([128, N], FP32, tag="rz", bufs=6, name=f"rzt{g}_{t}_{j}")
                        nc.scalar.activation(rzt[:], prz[:], AT.Sigmoid, bias=brz_sb[j][:])
                        rz.append(rzt)
                    H_new = []
                    for j in range(KM):
                        jj = 4 + j
                        pgi = ps_sm.tile([128, N], FP32, tag="sm", name=f"gin{g}_{t}_{j}")
                        for k in range(KM):
                            nc.tensor.matmul(pgi[:], wih_sb[k][:, jj * 128:(jj + 1) * 128],
                                             m_fin[k][:], start=(k == 0), stop=(k == KM - 1))
                        pgh = ps_sm.tile([128, N], FP32, tag="sm", name=f"ghn{g}_{t}_{j}")
                        for k in range(KM):
                            nc.tensor.matmul(pgh[:], whh_sb[k][:, jj * 128:(jj + 1) * 128],
                                             H_cur[k][:], start=(k == 0), stop=(k == KM - 1))
                        s1 = sb.tile([128, N], FP32, tag="s1", bufs=2, name=f"s1{g}_{t}_{j}")
                        nc.scalar.activation(s1[:], pgh[:], AT.Identity, bias=bhhn_sb[j][:])
                        s2 = sb.tile([128, N], FP32, tag="s2", bufs=2, name=f"s2{g}_{t}_{j}")
                        nc.vector.tensor_tensor(s2[:], rz[j][:], s1[:], op=OP.mult)
                        s3 = sb.tile([128, N], FP32, tag="s3", bufs=2, name=f"s3{g}_{t}_{j}")
                        nc.vector.tensor_tensor(s3[:], s2[:], pgi[:], op=OP.add)
                        nn = sb.tile([128, N], FP32, tag="nn", bufs=2, name=f"nn{g}_{t}_{j}")
                        nc.scalar.activation(nn[:], s3[:], AT.Tanh, bias=bihn_sb[j][:])
                        d1 = sb.tile([128, N], FP32, tag="d1", bufs=2, name=f"d1{g}_{t}_{j}")
                        nc.vector.tensor_tensor(d1[:], H_cur[j][:], nn[:], op=OP.subtract)
                        zd = sb.tile([128, N], FP32, tag="zd", bufs=2, name=f"zd{g}_{t}_{j}")
                        nc.vector.tensor_tensor(zd[:], rz[2 + j][:], d1[:], op=OP.mult)
                        hp = sb.tile([128, N], FP32, tag="hp", bufs=2, name=f"hp{g}_{t}_{j}")
                        nc.vector.tensor_tensor(hp[:], nn[:], zd[:], op=OP.add)
                        d2 = sb.tile([128, N], FP32, tag="d2", bufs=2, name=f"d2{g}_{t}_{j}")
                        nc.vector.tensor_tensor(d2[:], hp[:], H_cur[j][:], op=OP.subtract)
                        md = sb.tile([128, N], FP32, tag="md", bufs=2, name=f"md{g}_{t}_{j}")
                        nc.vector.tensor_tensor(md[:], mv_sb[:], d2[:], op=OP.mult)
                        hnw = sb.tile([128, N], F32R, tag=f"H{g}_{j}", bufs=2,
                                      name=f"H{g}_{t + 1}_{j}")
                        nc.vector.tensor_tensor(hnw[:], H_cur[j][:], md[:], op=OP.add)
                        H_new.append(hnw)
                    H_cur = H_new

                H_final[g] = H_cur
                if not last:
                    for c_ in range(4, NCH):
                        states[g + 1]["ph1a"](c_)
                    for c_ in range(NCH):
                        states[g + 1]["ph1b"](c_)

            # ---- pair gather (both graphs) ----
            for g in range(BG):
                H_cur = H_final[g]
                h_nm = sb.tile([N, M], F32R, tag="h_nm", bufs=2, name=f"hnm_t{g}")
                for j in range(KM):
                    pt = ps_sm.tile([N, 128], FP32, tag="sm", name=f"htr{g}_{j}")
                    nc.tensor.transpose(pt[:], H_cur[j][:].bitcast(FP32), id_sb[:, :])
                    nc.vector.tensor_copy(h_nm[:, j * 128:(j + 1) * 128], pt[:])
                gt = sb.tile([N, 128], F32R, tag="gt", bufs=2, name=f"gt{g}")
                nc.sync.dma_start(gt[:], gmat[g])
                for j in range(KM):
                    pci = ps_mm.tile([128, 128], FP32, tag="mm", name=f"ci{g}_{j}")
                    nc.tensor.matmul(pci[:], h_nm[:, j * 128:(j + 1) * 128], gt[:], **MMOP)
                    nc.vector.tensor_copy(ci_sb[j][:, g * 128:(g + 1) * 128], pci[:])

            # ================= phase 4: classifiers (batched) =================
            for key in ("lr", "cr", "mr"):
                od = cls_od[key]
                h1 = []
                for j in range(KH):
                    ph = ps_mm.tile([128, BG * 128], FP32, tag="mm", name=f"c1{key}_{j}")
                    for k in range(KM):
                        nc.tensor.matmul(ph[:], c1_sb[key][k][:, j * 128:(j + 1) * 128],
                                         ci_sb[k][:], start=(k == 0), stop=(k == KM - 1))
                    ht = sb.tile([128, BG * 128], F32R, tag=f"h1{key}", bufs=2,
                                 name=f"h1{key}_{j}")
                    nc.scalar.activation(ht[:], ph[:], AT.Relu, bias=cb1_sb[key][j][:])
                    h1.append(ht)
                po = ps_mm.tile([od, BG * 128], FP32, tag="mm", name=f"c2{key}")
                for k in range(KH):
                    nc.tensor.matmul(po[:], c2_sb[key][k][:], h1[k][:],
                                     start=(k == 0), stop=(k == KH - 1))
                osb = sb.tile([od, BG * 128], FP32, tag="osb", bufs=2, name=f"osb{key}")
                nc.scalar.activation(osb[:], po[:], AT.Identity, bias=cb2_sb[key][:])
                for g in range(BG):
                    ptr = ps_sm.tile([128, od], FP32, tag="sm", name=f"otr{key}_{g}")
                    nc.tensor.transpose(ptr[:], osb[:, g * 128:(g + 1) * 128],
                                        id_sb[0:od, 0:od])
                    og = sb.tile([128, od], FP32, tag="og", bufs=2, name=f"og{key}_{g}")
                    nc.vector.tensor_copy(og[:], ptr[:])
                    nc.sync.dma_start(outmap[key][g], og[:])

    nc.compile()
    return nc


def _get_runner():
    if "runner" in _CACHE:
        return _CACHE["runner"]
    import jax
    import numpy as np
    from jax.experimental.shard_map import shard_map
    from jax.sharding import Mesh, NamedSharding, PartitionSpec
    from concourse import mybir
    from concourse.bass2jax import (_bass_exec_p, install_neuronx_cc_hook,
                                    partition_id_tensor)

    nc = _build_program()
    _CACHE["nc"] = nc
    install_neuronx_cc_hook()

    pname = nc.partition_id_tensor.name if nc.partition_id_tensor else None
    in_names, out_names, out_avals, zero_outs = [], [], [], []
    for alloc in nc.m.functions[0].allocations:
        if not isinstance(alloc, mybir.MemoryLocationSet):
            continue
        name = alloc.memorylocations[0].name
        if alloc.kind == "ExternalInput":
            if name != pname:
                in_names.append(name)
        elif alloc.kind == "ExternalOutput":
            out_names.append(name)
            shape = tuple(alloc.tensor_shape)
            dtype = mybir.dt.np(alloc.dtype)
            out_avals.append(jax.core.ShapedArray(shape, dtype))
            zero_outs.append(np.zeros(shape, dtype))
    n_params = len(in_names)
    all_in_names = in_names + out_names
    if pname is not None:
        all_in_names = all_in_names + [pname]

    def _body(*args):
        operands = list(args)
        if pname is not None:
            operands.append(partition_id_tensor())
        outs = _bass_exec_p.bind(
            *operands,
            out_avals=tuple(out_avals),
            in_names=tuple(all_in_names),
            out_names=tuple(out_names),
            lowering_input_output_aliases=(),
            sim_require_finite=False,
            sim_require_nnan=False,
            nc=nc,
        )
        return tuple(outs)

    devices = jax.devices()[:NCORES]
    mesh = Mesh(np.asarray(devices), ("core",))
    n_all = n_params + len(zero_outs)
    sharded = jax.jit(
        shard_map(_body, mesh=mesh,
                  in_specs=(PartitionSpec("core"),) * n_all,
                  out_specs=(PartitionSpec("core"),) * len(out_names),
                  check_rep=False),
        keep_unused=True,
    )
    sharding = NamedSharding(mesh, PartitionSpec("core"))
    runner = dict(sharded=sharded, in_names=in_names, out_names=out_names,
                  out_avals=out_avals, zero_outs=zero_outs, sharding=sharding,
                  mesh=mesh)
    _CACHE["runner"] = runner
    return runner


def _pack_biases(inputs):
    f32 = np.float32
    cols = []

    def add(vec, chunks=None):
        v = np.asarray(vec, f32).ravel()
        n = (len(v) + 127) // 128 if chunks is None else chunks
        for k in range(n):
            c = np.zeros(128, f32)
            seg = v[k * 128:(k + 1) * 128]
            c[:len(seg)] = seg
            cols.append(c)

    be = np.asarray(inputs["b_edge_rs"], np.float64)
    add(np.asarray(inputs["Wm_edge"], np.float64) @ be)          # b_en: 2
    add(np.asarray(inputs["link_W1"], np.float64) @ be
        + np.asarray(inputs["link_b1"], np.float64))             # b1t0: 4
    add(inputs["b_node_rs"])                      # 2
    add(inputs["link_b1"])                        # 4
    add(inputs["link_b2"])                        # 4
    add(inputs["link_b3"])                        # 1
    add(inputs["bm"])                             # 2
    add(np.asarray(inputs["bih"], f32)[0:2 * M]
        + np.asarray(inputs["bhh"], f32)[0:2 * M])  # 4
    add(np.asarray(inputs["bih"], f32)[2 * M:])   # 2
    add(np.asarray(inputs["bhh"], f32)[2 * M:])   # 2
    for k in ("lr", "cr", "mr"):
        add(inputs[f"{k}_b1"])                    # 4 each
    for k in ("lr", "cr", "mr"):
        add(inputs[f"{k}_b2"], chunks=1)          # 1 each
    out = np.stack(cols, axis=1)
    assert out.shape[1] == NBIAS, out.shape
    return np.ascontiguousarray(out)


def _preprocess(inputs):
    """Host-side prep: per-core input dict values, each shaped [ncores*d0, ...]."""
    f32 = np.float32
    rsf = np.ascontiguousarray(inputs["relative_spatial_feature"], f32)
    cnf = np.ascontiguousarray(inputs["concatenated_node_features"], f32)
    num_obj = np.asarray(inputs["num_obj"])
    pairs = np.asarray(inputs["object_pairs"])

    T = lambda a: np.ascontiguousarray(np.asarray(a, f32).T)
    col = lambda a: np.ascontiguousarray(np.asarray(a, f32).reshape(-1, 1))

    per_graph_hnmask = np.zeros((B, 128, N), f32)
    per_graph_maskv = np.zeros((B, 128, N), f32)
    per_graph_mask2 = np.zeros((B, NE), f32)
    per_graph_G = np.zeros((B, N, 128), f32)
    for b in range(B):
        valid = (np.arange(N) < int(num_obj[b]))
        per_graph_hnmask[b, :, :] = np.where(valid, 0.0, NEG)[None, :]
        per_graph_maskv[b, :, :] = valid.astype(f32)[None, :]
        per_graph_mask2[b, :] = (valid[:, None] & valid[None, :]).astype(f32).ravel()
        gm = np.zeros((N, 128), f32)
        for p in range(128):
            gm[int(pairs[b, p, 0]), p] += 0.5
            gm[int(pairs[b, p, 1]), p] += 0.5
        per_graph_G[b] = gm

    shared = {
        "wen_t": np.ascontiguousarray(
            (np.asarray(inputs["Wm_edge"], np.float64)
             @ np.asarray(inputs["W_edge_rs"], np.float64)).T.astype(f32)),
        "w1e_t": np.ascontiguousarray(
            (np.asarray(inputs["link_W1"], np.float64)
             @ np.asarray(inputs["W_edge_rs"], np.float64)).T.astype(f32)),
        "wnr_t": T(inputs["W_node_rs"]),
        "w1_t": T(inputs["link_W1"]), "w2_t": T(inputs["link_W2"]),
        "w3_t": T(inputs["link_W3"]),
        "wmn_t": T(inputs["Wm_node"]),
        "wih_t": T(inputs["Wih"]), "whh_t": T(inputs["Whh"]),
        "lr1_t": T(inputs["lr_W1"]), "lr2_t": T(inputs["lr_W2"]),
        "cr1_t": T(inputs["cr_W1"]), "cr2_t": T(inputs["cr_W2"]),
        "mr1_t": T(inputs["mr_W1"]), "mr2_t": T(inputs["mr_W2"]),
        "ones": np.ones((1, 128), f32), "ident": np.eye(128, dtype=f32),
        "bias_all": _pack_biases(inputs),
    }

    concat = {}
    concat["rsf"] = rsf.reshape(B, NE, EF)          # [16*2? -> (8*2, NE, EF)]
    concat["cnf"] = cnf
    concat["hnmask"] = per_graph_hnmask
    concat["maskv"] = per_graph_maskv
    concat["mask2"] = per_graph_mask2
    concat["gmat"] = per_graph_G
    for k, v in shared.items():
        concat[k] = np.concatenate([v] * NCORES, axis=0)
    return concat


def _postprocess(out_map):
    lr = out_map["out_lr"].reshape(B, 128, 4)
    cr = out_map["out_cr"].reshape(B, 128, 6)
    mr = out_map["out_mr"].reshape(B, 128, 17)
    pa = out_map["out_pa"].reshape(B, N, N)
    return lr, cr, mr, pa


def _run_concat(concat):
    import jax
    r = _get_runner()
    args = [np.ascontiguousarray(concat[n]) for n in r["in_names"]]
    zeros = [np.zeros((NCORES * z.shape[0], *z.shape[1:]), z.dtype)
             for z in r["zero_outs"]]
    outs = r["sharded"](*args, *zeros)
    return {n: np.asarray(outs[i]) for i, n in enumerate(r["out_names"])}


def kernel(**inputs):
    concat = _preprocess(inputs)
    out_map = _run_concat(concat)
    return _postprocess(out_map)


if __name__ == "__main__":
    rng = np.random.default_rng(0)
    print("building...")
    _get_runner()
    print("built ok")
